# revision 5
# baseline (speedup 1.0000x reference)
"""Trainium2 Bass kernel for nn_CartographerPoseCorrector.

Strategy
--------
The reference refines, per (ego, nbr) pair, a 2x3 affine by scoring 7056
coarse + 729 fine candidate warps (bilinear grid-sample of nbr against ego)
and picking the argmax of each stage.

Device (8 NeuronCores, SPMD): for every coarse rotation theta (16 per pair,
sharded 4 per core; pairs split across core halves) compute integer-lag
moment-correlation surfaces on the TensorEngine:

    T_m[K,J] = sum_p mu_m(p) * ego[p] * nbr~[Yi(p)+J, Xi(p)+K]

for mu_m in {1, Xf, Yf, Xf*Yf}, lags J,K in [-24, 24).  (Yi,Xi / Yf,Xf are
the integer/fractional parts of the theta-warp sample positions; the
candidate-translation axis of the search grid collapses onto the lag axes.)

The device program uses fp8e4 DoubleRow matmuls: contraction = 256 (canvas-X
split into the Ko=2 interleave), 64 accumulation steps over image-row pairs,
two 2-unit matmul groups per step.  All operand layouts are prebuilt on the
host so the kernel is DMA -> 128 matmuls -> DMA out.

From these surfaces the host assembles, per candidate, the exact
no-carry-bilinear approximation of its score, keeps every candidate within a
safety margin of the max, exactly rescores that small set (and the 729 fine
candidates) in fp32, and takes the argmax - reproducing the reference's
selection exactly.  A tiny host argmax/gather finishes, per the sharding
hint.
"""

import math
import sys

import numpy as np

H = W = 128
THRESH = 0.3
TRANS_RANGE = 20.0
ROT_RANGE = 15.0
COARSE_STEP = 2.0
FINE_STEP = 0.5

# Device-kernel geometry (must match the Bass program)
NL = 48          # lags per axis
LMIN = -24       # lag range [LMIN, LMIN + NL)
NJ = NL + 1      # sliding J-window width
MROWS = 2 * NL   # psum rows: (t, slot)
OFFX = OFFY = 44 # image coord -> canvas coord offset
CY = 224         # canvas Y extent
CX = 256         # canvas X extent (2 Ko chunks of 128)
SRY = 176        # stored canvas-Y rows (window only touches y in [21, 196))
SRY0 = 21        # stored rows are S[.., SRY0 + y', ..]
U = 4            # units (theta-warps) per core
NGRP = 2         # matmul groups (2 units each)
NCOLS = NJ * 2 * 4  # 392 psum cols: (window pos, unit-in-group, moment)
N_CORES = 8

DELTA_COARSE = 280.0   # exact-rescore safety margin (measured errmax ~97)
RESCORE_CAP = 2200     # hard cap on rescored coarse candidates per pair

_NC = None


# ----------------------------------------------------------------------------
# host math (mirrors reference numerics in fp32 where it matters)
# ----------------------------------------------------------------------------

def _grid_1d(align_corners):
    if align_corners:
        xs = np.linspace(-1.0, 1.0, W, dtype=np.float32)
        ys = np.linspace(-1.0, 1.0, H, dtype=np.float32)
    else:
        xs = ((2.0 * np.arange(W, dtype=np.float32) + 1.0) / W - 1.0)
        ys = ((2.0 * np.arange(H, dtype=np.float32) + 1.0) / H - 1.0)
    return xs, ys


def _coarse_cands():
    dxs = np.arange(-TRANS_RANGE, TRANS_RANGE + 1e-3, COARSE_STEP, dtype=np.float32)
    drs = np.arange(-ROT_RANGE, ROT_RANGE + 1e-3, COARSE_STEP, dtype=np.float32)
    gdx, gdy, gdr = np.meshgrid(dxs, dxs, drs, indexing="ij")
    return np.stack([gdx.ravel(), gdy.ravel(), gdr.ravel()], axis=1)


def _fine_cands(cp):
    off = np.arange(-COARSE_STEP, COARSE_STEP + 1e-3, FINE_STEP, dtype=np.float32)
    gdx, gdy, gdr = np.meshgrid(cp[0] + off, cp[1] + off, cp[2] + off, indexing="ij")
    return np.stack([gdx.ravel(), gdy.ravel(), gdr.ravel()], axis=1)


def _cand_affines(cands, base_2x3):
    dx, dy, dr = cands[:, 0], cands[:, 1], cands[:, 2]
    tx = (2.0 * dx / max(W - 1, 1)).astype(np.float32)
    ty = (2.0 * dy / max(H - 1, 1)).astype(np.float32)
    th = (dr * np.float32(math.pi / 180.0)).astype(np.float32)
    c, s = np.cos(th), np.sin(th)
    z, o = np.zeros_like(c), np.ones_like(c)
    delta = np.stack([c, -s, tx, s, c, ty, z, z, o], axis=-1).reshape(-1, 3, 3)
    base3 = np.concatenate([base_2x3, np.array([[0, 0, 1]], np.float32)], axis=0)
    return np.einsum("ij,njk->nik", base3.astype(np.float32), delta.astype(np.float32))[
        :, :2, :
    ].astype(np.float32)


def _pad_nbr(nbr_c, padb=8):
    out = np.zeros((H + 2 * padb, W + 2 * padb), np.float32)
    out[padb : padb + H, padb : padb + W] = nbr_c
    return out


def _exact_scores(ego_c, nbrP, affs, align_corners, padb=8, chunk=16):
    """Exact fp32 bilinear grid-sample scores for candidate affines [n,2,3]."""
    xs, ys = _grid_1d(align_corners)
    gx = np.broadcast_to(xs[None, :], (H, W)).ravel().astype(np.float32)
    gy = np.broadcast_to(ys[:, None], (H, W)).ravel().astype(np.float32)
    flat = nbrP.ravel()
    Wp = nbrP.shape[1]
    if align_corners:
        scx, ox = np.float32(0.5 * (W - 1)), np.float32(0.5 * (W - 1))
        scy, oy = np.float32(0.5 * (H - 1)), np.float32(0.5 * (H - 1))
    else:
        scx, ox = np.float32(0.5 * W), np.float32(0.5 * W - 0.5)
        scy, oy = np.float32(0.5 * H), np.float32(0.5 * H - 0.5)
    ego_f = ego_c.ravel().astype(np.float32)
    N = len(affs)
    out = np.empty(N, np.float32)
    for s0 in range(0, N, chunk):
        A = affs[s0 : s0 + chunk].astype(np.float32)
        n = len(A)
        ix = np.multiply.outer(A[:, 0, 0], gx)
        ix += np.multiply.outer(A[:, 0, 1], gy)
        ix += A[:, 0, 2, None]
        ix *= scx
        ix += ox
        iy = np.multiply.outer(A[:, 1, 0], gx)
        iy += np.multiply.outer(A[:, 1, 1], gy)
        iy += A[:, 1, 2, None]
        iy *= scy
        iy += oy
        x0 = np.floor(ix)
        y0 = np.floor(iy)
        wx = ix - x0
        wy = iy - y0
        xi = x0.astype(np.int32)
        xi += padb
        np.clip(xi, 0, Wp - 2, out=xi)
        yi = y0.astype(np.int32)
        yi += padb
        np.clip(yi, 0, Wp - 2, out=yi)
        base = yi
        base *= Wp
        base += xi
        b00 = flat[base]
        b01 = flat[base + 1]
        b10 = flat[base + Wp]
        b11 = flat[base + Wp + 1]
        top = (1.0 - wx) * b00
        top += wx * b01
        bot = (1.0 - wx) * b10
        bot += wx * b11
        val = (1.0 - wy) * top
        val += wy * bot
        out[s0 : s0 + n] = val @ ego_f
    return out


def _theta_warp_fields(base_2x3, dr, align_corners):
    """Pixel-coord sample positions of the theta-only warp (dx=dy=0)."""
    th = np.float32(dr) * np.float32(math.pi / 180.0)
    c, s = np.cos(th, dtype=np.float32), np.sin(th, dtype=np.float32)
    delta = np.array([[c, -s, 0], [s, c, 0], [0, 0, 1]], np.float32)
    base3 = np.concatenate([base_2x3, [[0, 0, 1]]], 0).astype(np.float32)
    aff = (base3 @ delta)[:2]
    xs, ys = _grid_1d(align_corners)
    gx = aff[0, 0] * xs[None, :] + aff[0, 1] * ys[:, None] + aff[0, 2]
    gy = aff[1, 0] * xs[None, :] + aff[1, 1] * ys[:, None] + aff[1, 2]
    if align_corners:
        ix = (gx + 1.0) * (0.5 * (W - 1))
        iy = (gy + 1.0) * (0.5 * (H - 1))
    else:
        ix = gx * (0.5 * W) + (0.5 * W - 0.5)
        iy = gy * (0.5 * H) + (0.5 * H - 0.5)
    return ix.astype(np.float32), iy.astype(np.float32)


def _trans_shifts(base_2x3, cands, align_corners):
    """Pixel-space shifts (ux, uy) each candidate translation adds."""
    B2 = base_2x3[:2, :2].astype(np.float32)
    tx = (2.0 * cands[:, 0] / (W - 1)).astype(np.float32)
    ty = (2.0 * cands[:, 1] / (H - 1)).astype(np.float32)
    if align_corners:
        sx, sy = 0.5 * (W - 1), 0.5 * (H - 1)
    else:
        sx, sy = 0.5 * W, 0.5 * H
    ux = (B2[0, 0] * tx + B2[0, 1] * ty) * np.float32(sx)
    uy = (B2[1, 0] * tx + B2[1, 1] * ty) * np.float32(sy)
    return ux, uy


def _build_splats(ego_c, ix, iy):
    """Moment splat canvases [4, CY, CX] f32, or None if out of range."""
    Xi = np.floor(ix)
    Yi = np.floor(iy)
    Xf = (ix - Xi).astype(np.float32)
    Yf = (iy - Yi).astype(np.float32)
    Xi = Xi.astype(np.int64)
    Yi = Yi.astype(np.int64)
    if (
        Xi.min() < -OFFX
        or Xi.max() >= CX - OFFX
        or Yi.min() < -OFFY
        or Yi.max() >= CY - OFFY
    ):
        return None
    S = np.zeros((4, CY, CX), np.float32)
    flatidx = ((Yi + OFFY) * CX + (Xi + OFFX)).ravel()
    nbins = CY * CX
    for m, mu in enumerate((None, Xf, Yf, Xf * Yf)):
        wgt = ego_c if mu is None else mu * ego_c
        S[m] = (
            np.bincount(flatidx, weights=wgt.ravel().astype(np.float64), minlength=nbins)
            .reshape(CY, CX)
            .astype(np.float32)
        )
    return S


def _build_wq(nbr_c):
    """Stationary windows WQ[c, i, h, 2t+slot] = nbr[2i+slot, c+128h+t-68]."""
    WQ = np.zeros((128, 64, 2, MROWS), np.float32)
    c = np.arange(128)[:, None, None]
    h = np.arange(2)[None, :, None]
    t = np.arange(NL)[None, None, :]
    v = c + 128 * h + t - 68
    valid = (v >= 0) & (v < W)
    vc = np.clip(v, 0, W - 1)
    for slot in range(2):
        # [c, h, t] gather per row y -> place at [c, i, h, slot::2]
        rows = nbr_c[slot::2, :]  # [64, W]
        vals = np.where(valid[None], rows[:, vc], 0.0)  # [64, c, h, t]
        WQ[:, :, :, slot::2] = vals.transpose(1, 0, 2, 3)
    return WQ


def _assemble_approx(T, base_2x3, cands, align_corners):
    """Approx scores for one theta's candidates from its surface T [NL, 4, NL].

    Returns None if any candidate's lag falls outside the computed window
    (caller falls back to the exact host path)."""
    ux, uy = _trans_shifts(base_2x3, cands, align_corners)
    Ui = np.floor(ux).astype(np.int64)
    Ufx = (ux - Ui).astype(np.float32)
    Vi = np.floor(uy).astype(np.int64)
    Ufy = (uy - Vi).astype(np.float32)
    if (
        Ui.min() < LMIN
        or Ui.max() + 1 >= LMIN + NL
        or Vi.min() < LMIN
        or Vi.max() + 1 >= LMIN + NL
    ):
        return None
    out = np.zeros(len(cands), np.float32)
    for j in (0, 1):
        ay = np.where(j, Ufy, 1.0 - Ufy).astype(np.float32)
        by = 1.0 if j else -1.0
        Jp = Vi + j - LMIN
        for k in (0, 1):
            ax = np.where(k, Ufx, 1.0 - Ufx).astype(np.float32)
            bx = 1.0 if k else -1.0
            Kp = Ui + k - LMIN
            out += ax * ay * T[Kp, 0, Jp]
            out += bx * ay * T[Kp, 1, Jp]
            out += ax * by * T[Kp, 2, Jp]
            out += bx * by * T[Kp, 3, Jp]
    return out


def _combine_T(psum):
    """psum [MROWS, NJ, 2, 4] -> T[u][K(NL), m(4), J(NL)]."""
    T = np.zeros((2, NL, 4, NL), np.float32)
    J = np.arange(LMIN, LMIN + NL)
    w0 = 23 - J
    w1 = 24 - J
    for u in range(2):
        p0 = psum[0::2, :, u, :][:, w0, :]  # [t, J, m]
        p1 = psum[1::2, :, u, :][:, w1, :]
        T[u] = (p0 + p1).transpose(0, 2, 1)
    return T


# ----------------------------------------------------------------------------
# device program
# ----------------------------------------------------------------------------

N_WQ_CHUNKS = 4
N_SR_CHUNKS = 4
N_WARMUP_MM = 20


def _get_nc():
    global _NC
    if _NC is not None:
        return _NC
    sys.path.insert(0, "/opt/trn_rl_repo")
    from contextlib import ExitStack

    import concourse.bass as bass
    import concourse.mybir as mybir
    import concourse.tile as tile
    from concourse import bacc

    nc = bacc.Bacc("TRN2", target_bir_lowering=False, debug=False)
    wq = nc.declare_dram_parameter("wq", [128, 64, 2, MROWS], mybir.dt.float8e4, isOutput=False)
    sra = nc.declare_dram_parameter("sra", [128, 2, SRY, 8], mybir.dt.float8e4, isOutput=False)
    srb = nc.declare_dram_parameter("srb", [128, 2, SRY, 8], mybir.dt.float8e4, isOutput=False)
    tout = nc.declare_dram_parameter("tout", [NGRP, MROWS, NCOLS], mybir.dt.float32, isOutput=True)
    wq_h = wq.tensor if isinstance(wq, bass.AP) else wq
    sra_h = sra.tensor if isinstance(sra, bass.AP) else sra
    srb_h = srb.tensor if isinstance(srb, bass.AP) else srb
    tout_h = tout.tensor if isinstance(tout, bass.AP) else tout

    DR = mybir.MatmulPerfMode.DoubleRow

    with ExitStack() as ctx:
        tc = ctx.enter_context(tile.TileContext(nc))
        pool = ctx.enter_context(tc.tile_pool(name="persist", bufs=1))
        psum_pool = ctx.enter_context(tc.tile_pool(name="psum", bufs=1, space="PSUM"))

        wq_t = pool.tile([128, 64, 2, MROWS], mybir.dt.float8e4)
        sra_t = pool.tile([128, 2, SRY, 8], mybir.dt.float8e4)
        srb_t = pool.tile([128, 2, SRY, 8], mybir.dt.float8e4)

        # PE warm-up: zero-filled dummy DoubleRow matmuls keep the HAM busy
        # while input DMAs land, so the real loop starts at 2.4 GHz.
        wdum = pool.tile([128, 2, 16], mybir.dt.float8e4)
        sdum = pool.tile([128, 2, 128], mybir.dt.float8e4)
        pdum = psum_pool.tile([16, 128], mybir.dt.float32, name="pdum", tag="pdum")
        nc.vector.memset(wdum[:], 0.0)
        nc.vector.memset(sdum[:], 0.0)
        for k in range(N_WARMUP_MM):
            nc.tensor.matmul(pdum[:], wdum[:], sdum[:], start=True, stop=True,
                             perf_mode=DR, skip_group_check=True)

        # chunked input DMAs (slice-level deps let matmuls start early)
        wq_csz = 64 // N_WQ_CHUNKS
        sr_csz = (SRY + N_SR_CHUNKS - 1) // N_SR_CHUNKS
        for k in range(max(N_WQ_CHUNKS, N_SR_CHUNKS)):
            if k < N_WQ_CHUNKS:
                i0 = k * wq_csz
                src = bass.AP(tensor=wq_h, offset=i0 * 2 * MROWS,
                              ap=[[64 * 2 * MROWS, 128], [1, wq_csz * 2 * MROWS]])
                nc.sync.dma_start(out=wq_t[:, i0:i0 + wq_csz], in_=src)
            if k < N_SR_CHUNKS:
                y0 = k * sr_csz
                y1 = min(SRY, y0 + sr_csz)
                for srh, srt in ((sra_h, sra_t), (srb_h, srb_t)):
                    src = bass.AP(tensor=srh, offset=y0 * 8,
                                  ap=[[2 * SRY * 8, 128], [SRY * 8, 2], [1, (y1 - y0) * 8]])
                    nc.sync.dma_start(out=srt[:, :, y0:y1], in_=src)

        psums = [
            psum_pool.tile([MROWS, NCOLS], mybir.dt.float32, name=f"psum{g}", tag=f"psum{g}")
            for g in range(NGRP)
        ]
        for i in range(64):
            lhsT = wq_t[:, i]  # [128, 2, MROWS]
            for g, srt in enumerate((sra_t, srb_t)):
                rhs = srt[:, :, 2 * i:2 * i + NJ, :]  # [128, 2, NJ, 8]
                nc.tensor.matmul(psums[g][:], lhsT, rhs,
                                 start=(i == 0), stop=(i == 63), perf_mode=DR)

        for g in range(NGRP):
            stg = pool.tile([MROWS, NCOLS], mybir.dt.float32, name=f"stg{g}", tag=f"stg{g}")
            nc.scalar.copy(stg[:], psums[g][:])
            dst = bass.AP(tensor=tout_h, offset=g * MROWS * NCOLS,
                          ap=[[NCOLS, MROWS], [1, NCOLS]])
            nc.sync.dma_start(out=dst, in_=stg[:])
    nc.compile()
    _NC = nc
    return nc


def _run_device(in_maps, trace=False):
    sys.path.insert(0, "/opt/trn_rl_repo")
    import ml_dtypes
    from concourse.bass_utils import run_bass_kernel_spmd

    fp8 = ml_dtypes.float8_e4m3
    maps = [
        {
            "wq": np.ascontiguousarray(m["wq"]).astype(fp8),
            "sra": np.ascontiguousarray(m["sra"]).astype(fp8),
            "srb": np.ascontiguousarray(m["srb"]).astype(fp8),
        }
        for m in in_maps
    ]
    res = run_bass_kernel_spmd(_get_nc(), maps, core_ids=list(range(len(maps))),
                               trace=trace)
    touts = [
        r["tout"].astype(np.float32).reshape(NGRP, MROWS, NJ, 2, 4) for r in res.results
    ]
    if trace:
        return touts, res
    return touts


# ----------------------------------------------------------------------------
# pipeline
# ----------------------------------------------------------------------------

def _refine_pair_host_only(ego_c, nbr_c, base, align_corners):
    """Pure-host exact fallback (pathological inputs only)."""
    nbrP = _pad_nbr(nbr_c)
    cands = _coarse_cands()
    sc = _exact_scores(ego_c, nbrP, _cand_affines(cands, base), align_corners)
    bi = int(np.argmax(sc))
    cp = cands[bi] if sc[bi] > 1e-5 else np.zeros(3, np.float32)
    if np.all(cp == 0.0):
        return base
    fc = _fine_cands(cp)
    affs_f = _cand_affines(fc, base)
    sf = _exact_scores(ego_c, nbrP, affs_f, align_corners)
    bif = int(np.argmax(sf))
    return affs_f[bif] if sf[bif] > 1e-5 else base


def _finish_pair(ego_c, nbrP, base, cands, approx, align_corners):
    """Adaptive exact rescore of the approx-selected coarse set -> cp."""
    thresh = approx.max() - DELTA_COARSE
    sel = np.where(approx >= thresh)[0]
    if len(sel) > RESCORE_CAP:
        sel = sel[np.argsort(approx[sel])[::-1][:RESCORE_CAP]]
    if len(sel) < 48:
        sel = np.argsort(approx)[::-1][:48]
    affs = _cand_affines(cands[sel], base)
    sc = _exact_scores(ego_c, nbrP, affs, align_corners)
    bi_local = int(np.argmax(sc))
    bi = int(sel[bi_local])
    ok = sc[bi_local] > 1e-5
    cp = cands[bi] if ok else np.zeros(3, np.float32)
    return cp


def kernel(occ_map, record_len, affine_matrix, align_corners):
    occ = np.asarray(occ_map, dtype=np.float32)
    rl = np.asarray(record_len).reshape(-1)
    aff_in = np.asarray(affine_matrix)
    out_dtype = aff_in.dtype
    refined = aff_in.astype(np.float32).copy()
    ac = bool(np.asarray(align_corners))

    # pair list exactly as the reference builds it
    pairs = []
    idx = 0
    for b in range(len(rl)):
        n_agents = int(rl[b])
        grp0 = idx
        idx += n_agents
        if n_agents <= 1:
            continue
        for n in range(1, n_agents):
            pairs.append((b, n, grp0, grp0 + n))
    if not pairs:
        return refined.astype(out_dtype)

    device_ok = (
        len(pairs) <= 2
        and all(
            b < refined.shape[0] and n < refined.shape[2] and nb < occ.shape[0]
            for (b, n, _, nb) in pairs
        )
    )

    pair_data = []
    for (b, n, ei, ni) in pairs:
        # mimic jax OOB semantics: clip gather indices, drop OOB scatters
        ei = min(ei, occ.shape[0] - 1)
        ni = min(ni, occ.shape[0] - 1)
        ego = occ[ei, 0]
        nbr = occ[ni, 0]
        ego_c = np.where(ego > THRESH, ego, 0.0).astype(np.float32)
        nbr_c = np.where(nbr > THRESH, nbr, 0.0).astype(np.float32)
        base = refined[b, 0, n].astype(np.float32)
        pair_data.append(
            {
                "b": min(b, refined.shape[0] - 1),
                "n": n,
                "ego_c": ego_c,
                "nbr_c": nbr_c,
                "nbrP": _pad_nbr(nbr_c),
                "base": base,
            }
        )

    cands = _coarse_cands()
    drs = np.unique(cands[:, 2])  # 16 rotations
    by_dr = {float(dr): np.where(cands[:, 2] == dr)[0] for dr in drs}

    # build device inputs: 16 theta-units per pair, 4 per core; cores 0-3 pair0,
    # cores 4-7 pair1
    use_device = device_ok
    unit_map = {}  # (core, u) -> (pair_idx, dr)
    in_maps = None
    if use_device:
        zero_wq = np.zeros((128, 64, 2, MROWS), np.float32)
        zero_sr = np.zeros((128, 2, SRY, 8), np.float32)
        in_maps = []
        splat_fail = False
        wq_cache = {}
        for core in range(N_CORES):
            pi = core // 4
            if pi >= len(pair_data):
                in_maps.append({"wq": zero_wq, "sra": zero_sr, "srb": zero_sr})
                continue
            pd = pair_data[pi]
            if pi not in wq_cache:
                wq_cache[pi] = _build_wq(pd["nbr_c"])
            srs = []
            for g in range(NGRP):
                SR = np.zeros((128, 2, SRY, 8), np.float32)
                for ug in range(2):
                    u = 2 * g + ug
                    th_idx = 4 * (core % 4) + u
                    dr = float(drs[th_idx])
                    ix, iy = _theta_warp_fields(pd["base"], dr, ac)
                    S = _build_splats(pd["ego_c"], ix, iy)
                    if S is None:
                        splat_fail = True
                        break
                    # SR[c, h, y', u, m] = S[m, SRY0 + y', c + 128h]
                    blk = S[:, SRY0:SRY0 + SRY, :].transpose(2, 1, 0)  # [cx, y', m]
                    SR[:, :, :, 4 * ug:4 * ug + 4] = (
                        blk.reshape(2, 128, SRY, 4).transpose(1, 0, 2, 3)
                    )
                    unit_map[(core, u)] = (pi, dr)
                if splat_fail:
                    break
                srs.append(SR)
            if splat_fail:
                break
            in_maps.append({"wq": wq_cache[pi], "sra": srs[0], "srb": srs[1]})
        if splat_fail:
            use_device = False

    if use_device:
        try:
            global _LAST_IN_MAPS
            _LAST_IN_MAPS = in_maps
            touts = _run_device(in_maps)
        except Exception:
            use_device = False

    for pi, pd in enumerate(pair_data):
        base = pd["base"]
        pair_device = use_device
        approx = None
        if pair_device:
            approx = np.empty(len(cands), np.float32)
            for core in range(4 * pi, 4 * pi + 4):
                for g in range(NGRP):
                    Tg = _combine_T(touts[core][g])
                    for ug in range(2):
                        u = 2 * g + ug
                        key = (core, u)
                        if key not in unit_map:
                            continue
                        _, dr = unit_map[key]
                        sel = by_dr[dr]
                        a = _assemble_approx(Tg[ug], base, cands[sel], ac)
                        if a is None:
                            pair_device = False
                            break
                        approx[sel] = a
                    if not pair_device:
                        break
                if not pair_device:
                    break
        if pair_device:
            cp = _finish_pair(pd["ego_c"], pd["nbrP"], base, cands, approx, ac)
            if np.all(cp == 0.0):
                new_aff = base
            else:
                fc = _fine_cands(cp)
                affs_f = _cand_affines(fc, base)
                sf = _exact_scores(pd["ego_c"], pd["nbrP"], affs_f, ac)
                bif = int(np.argmax(sf))
                new_aff = affs_f[bif] if sf[bif] > 1e-5 else base
        else:
            new_aff = _refine_pair_host_only(pd["ego_c"], pd["nbr_c"], base, ac)
        if pd["n"] < refined.shape[2] and pd["b"] < refined.shape[0]:
            refined[pd["b"], 0, pd["n"]] = new_aff

    return refined.astype(out_dtype)


# revision 8
# speedup vs baseline: 1.0481x; 1.0481x over previous
"""Trainium2 Bass kernel for nn_CartographerPoseCorrector.

Strategy
--------
The reference refines, per (ego, nbr) pair, a 2x3 affine by scoring 7056
coarse + 729 fine candidate warps (bilinear grid-sample of nbr against ego)
and picking the argmax of each stage.

Device (8 NeuronCores, SPMD): for every coarse rotation theta (16 per pair,
sharded 4 per core; pairs split across core halves) compute integer-lag
moment-correlation surfaces on the TensorEngine:

    T_m[K,J] = sum_p mu_m(p) * ego[p] * nbr~[Yi(p)+J, Xi(p)+K]

for mu_m in {1, Xf, Yf, Xf*Yf}, lags J,K in [-24, 24).  (Yi,Xi / Yf,Xf are
the integer/fractional parts of the theta-warp sample positions; the
candidate-translation axis of the search grid collapses onto the lag axes.)

The device program uses fp8e4 DoubleRow matmuls: contraction = 256 (canvas-X
split into the Ko=2 interleave), 64 accumulation steps over image-row pairs,
two 2-unit matmul groups per step.  All operand layouts are prebuilt on the
host so the kernel is DMA -> 128 matmuls -> DMA out.

From these surfaces the host assembles, per candidate, the exact
no-carry-bilinear approximation of its score, keeps every candidate within a
safety margin of the max, exactly rescores that small set (and the 729 fine
candidates) in fp32, and takes the argmax - reproducing the reference's
selection exactly.  A tiny host argmax/gather finishes, per the sharding
hint.
"""

import math
import sys

import numpy as np

H = W = 128
THRESH = 0.3
TRANS_RANGE = 20.0
ROT_RANGE = 15.0
COARSE_STEP = 2.0
FINE_STEP = 0.5

# Device-kernel geometry (must match the Bass program)
NL = 48          # lags per axis
LMIN = -24       # lag range [LMIN, LMIN + NL)
NJ = NL + 1      # sliding J-window width
MROWS = 2 * NL   # psum rows: (t, slot)
OFFX = OFFY = 44 # image coord -> canvas coord offset
CY = 224         # canvas Y extent
CX = 256         # canvas X extent (2 Ko chunks of 128)
SRY = 176        # stored canvas-Y rows (window only touches y in [21, 196))
SRY0 = 21        # stored rows are S[.., SRY0 + y', ..]
U = 4            # units (theta-warps) per core
NGRP = 2         # matmul groups (2 units each)
NCOLS = NJ * 2 * 4  # 392 psum cols: (window pos, unit-in-group, moment)
N_CORES = 8

DELTA_COARSE = 280.0   # exact-rescore safety margin (measured errmax ~97)
RESCORE_CAP = 2200     # hard cap on rescored coarse candidates per pair

_NC = None


# ----------------------------------------------------------------------------
# host math (mirrors reference numerics in fp32 where it matters)
# ----------------------------------------------------------------------------

def _grid_1d(align_corners):
    if align_corners:
        xs = np.linspace(-1.0, 1.0, W, dtype=np.float32)
        ys = np.linspace(-1.0, 1.0, H, dtype=np.float32)
    else:
        xs = ((2.0 * np.arange(W, dtype=np.float32) + 1.0) / W - 1.0)
        ys = ((2.0 * np.arange(H, dtype=np.float32) + 1.0) / H - 1.0)
    return xs, ys


def _coarse_cands():
    dxs = np.arange(-TRANS_RANGE, TRANS_RANGE + 1e-3, COARSE_STEP, dtype=np.float32)
    drs = np.arange(-ROT_RANGE, ROT_RANGE + 1e-3, COARSE_STEP, dtype=np.float32)
    gdx, gdy, gdr = np.meshgrid(dxs, dxs, drs, indexing="ij")
    return np.stack([gdx.ravel(), gdy.ravel(), gdr.ravel()], axis=1)


def _fine_cands(cp):
    off = np.arange(-COARSE_STEP, COARSE_STEP + 1e-3, FINE_STEP, dtype=np.float32)
    gdx, gdy, gdr = np.meshgrid(cp[0] + off, cp[1] + off, cp[2] + off, indexing="ij")
    return np.stack([gdx.ravel(), gdy.ravel(), gdr.ravel()], axis=1)


def _cand_affines(cands, base_2x3):
    dx, dy, dr = cands[:, 0], cands[:, 1], cands[:, 2]
    tx = (2.0 * dx / max(W - 1, 1)).astype(np.float32)
    ty = (2.0 * dy / max(H - 1, 1)).astype(np.float32)
    th = (dr * np.float32(math.pi / 180.0)).astype(np.float32)
    c, s = np.cos(th), np.sin(th)
    z, o = np.zeros_like(c), np.ones_like(c)
    delta = np.stack([c, -s, tx, s, c, ty, z, z, o], axis=-1).reshape(-1, 3, 3)
    base3 = np.concatenate([base_2x3, np.array([[0, 0, 1]], np.float32)], axis=0)
    return np.einsum("ij,njk->nik", base3.astype(np.float32), delta.astype(np.float32))[
        :, :2, :
    ].astype(np.float32)


def _pad_nbr(nbr_c, padb=8):
    out = np.zeros((H + 2 * padb, W + 2 * padb), np.float32)
    out[padb : padb + H, padb : padb + W] = nbr_c
    return out


def _exact_scores(ego_c, nbrP, affs, align_corners, padb=8, chunk=16):
    """Exact fp32 bilinear grid-sample scores for candidate affines [n,2,3]."""
    xs, ys = _grid_1d(align_corners)
    gx = np.broadcast_to(xs[None, :], (H, W)).ravel().astype(np.float32)
    gy = np.broadcast_to(ys[:, None], (H, W)).ravel().astype(np.float32)
    flat = nbrP.ravel()
    Wp = nbrP.shape[1]
    if align_corners:
        scx, ox = np.float32(0.5 * (W - 1)), np.float32(0.5 * (W - 1))
        scy, oy = np.float32(0.5 * (H - 1)), np.float32(0.5 * (H - 1))
    else:
        scx, ox = np.float32(0.5 * W), np.float32(0.5 * W - 0.5)
        scy, oy = np.float32(0.5 * H), np.float32(0.5 * H - 0.5)
    ego_f = ego_c.ravel().astype(np.float32)
    N = len(affs)
    out = np.empty(N, np.float32)
    for s0 in range(0, N, chunk):
        A = affs[s0 : s0 + chunk].astype(np.float32)
        n = len(A)
        ix = np.multiply.outer(A[:, 0, 0], gx)
        ix += np.multiply.outer(A[:, 0, 1], gy)
        ix += A[:, 0, 2, None]
        ix *= scx
        ix += ox
        iy = np.multiply.outer(A[:, 1, 0], gx)
        iy += np.multiply.outer(A[:, 1, 1], gy)
        iy += A[:, 1, 2, None]
        iy *= scy
        iy += oy
        x0 = np.floor(ix)
        y0 = np.floor(iy)
        wx = ix - x0
        wy = iy - y0
        xi = x0.astype(np.int32)
        xi += padb
        np.clip(xi, 0, Wp - 2, out=xi)
        yi = y0.astype(np.int32)
        yi += padb
        np.clip(yi, 0, Wp - 2, out=yi)
        base = yi
        base *= Wp
        base += xi
        b00 = flat[base]
        b01 = flat[base + 1]
        b10 = flat[base + Wp]
        b11 = flat[base + Wp + 1]
        top = (1.0 - wx) * b00
        top += wx * b01
        bot = (1.0 - wx) * b10
        bot += wx * b11
        val = (1.0 - wy) * top
        val += wy * bot
        out[s0 : s0 + n] = val @ ego_f
    return out


def _theta_warp_fields(base_2x3, dr, align_corners):
    """Pixel-coord sample positions of the theta-only warp (dx=dy=0)."""
    th = np.float32(dr) * np.float32(math.pi / 180.0)
    c, s = np.cos(th, dtype=np.float32), np.sin(th, dtype=np.float32)
    delta = np.array([[c, -s, 0], [s, c, 0], [0, 0, 1]], np.float32)
    base3 = np.concatenate([base_2x3, [[0, 0, 1]]], 0).astype(np.float32)
    aff = (base3 @ delta)[:2]
    xs, ys = _grid_1d(align_corners)
    gx = aff[0, 0] * xs[None, :] + aff[0, 1] * ys[:, None] + aff[0, 2]
    gy = aff[1, 0] * xs[None, :] + aff[1, 1] * ys[:, None] + aff[1, 2]
    if align_corners:
        ix = (gx + 1.0) * (0.5 * (W - 1))
        iy = (gy + 1.0) * (0.5 * (H - 1))
    else:
        ix = gx * (0.5 * W) + (0.5 * W - 0.5)
        iy = gy * (0.5 * H) + (0.5 * H - 0.5)
    return ix.astype(np.float32), iy.astype(np.float32)


def _trans_shifts(base_2x3, cands, align_corners):
    """Pixel-space shifts (ux, uy) each candidate translation adds."""
    B2 = base_2x3[:2, :2].astype(np.float32)
    tx = (2.0 * cands[:, 0] / (W - 1)).astype(np.float32)
    ty = (2.0 * cands[:, 1] / (H - 1)).astype(np.float32)
    if align_corners:
        sx, sy = 0.5 * (W - 1), 0.5 * (H - 1)
    else:
        sx, sy = 0.5 * W, 0.5 * H
    ux = (B2[0, 0] * tx + B2[0, 1] * ty) * np.float32(sx)
    uy = (B2[1, 0] * tx + B2[1, 1] * ty) * np.float32(sy)
    return ux, uy


def _build_splats(ego_c, ix, iy):
    """Moment splat canvases [4, CY, CX] f32, or None if out of range."""
    Xi = np.floor(ix)
    Yi = np.floor(iy)
    Xf = (ix - Xi).astype(np.float32)
    Yf = (iy - Yi).astype(np.float32)
    Xi = Xi.astype(np.int64)
    Yi = Yi.astype(np.int64)
    if (
        Xi.min() < -OFFX
        or Xi.max() >= CX - OFFX
        or Yi.min() < -OFFY
        or Yi.max() >= CY - OFFY
    ):
        return None
    S = np.zeros((4, CY, CX), np.float32)
    flatidx = ((Yi + OFFY) * CX + (Xi + OFFX)).ravel()
    nbins = CY * CX
    for m, mu in enumerate((None, Xf, Yf, Xf * Yf)):
        wgt = ego_c if mu is None else mu * ego_c
        S[m] = (
            np.bincount(flatidx, weights=wgt.ravel().astype(np.float64), minlength=nbins)
            .reshape(CY, CX)
            .astype(np.float32)
        )
    return S


def _build_wq(nbr_c):
    """Stationary windows WQ[c, i, h, 2t+slot] = nbr[2i+slot, c+128h+t-68]."""
    WQ = np.zeros((128, 64, 2, MROWS), np.float32)
    c = np.arange(128)[:, None, None]
    h = np.arange(2)[None, :, None]
    t = np.arange(NL)[None, None, :]
    v = c + 128 * h + t - 68
    valid = (v >= 0) & (v < W)
    vc = np.clip(v, 0, W - 1)
    for slot in range(2):
        # [c, h, t] gather per row y -> place at [c, i, h, slot::2]
        rows = nbr_c[slot::2, :]  # [64, W]
        vals = np.where(valid[None], rows[:, vc], 0.0)  # [64, c, h, t]
        WQ[:, :, :, slot::2] = vals.transpose(1, 0, 2, 3)
    return WQ


def _assemble_approx(T, base_2x3, cands, align_corners):
    """Approx scores for one theta's candidates from its surface T [NL, 4, NL].

    Returns None if any candidate's lag falls outside the computed window
    (caller falls back to the exact host path)."""
    ux, uy = _trans_shifts(base_2x3, cands, align_corners)
    Ui = np.floor(ux).astype(np.int64)
    Ufx = (ux - Ui).astype(np.float32)
    Vi = np.floor(uy).astype(np.int64)
    Ufy = (uy - Vi).astype(np.float32)
    if (
        Ui.min() < LMIN
        or Ui.max() + 1 >= LMIN + NL
        or Vi.min() < LMIN
        or Vi.max() + 1 >= LMIN + NL
    ):
        return None
    out = np.zeros(len(cands), np.float32)
    for j in (0, 1):
        ay = np.where(j, Ufy, 1.0 - Ufy).astype(np.float32)
        by = 1.0 if j else -1.0
        Jp = Vi + j - LMIN
        for k in (0, 1):
            ax = np.where(k, Ufx, 1.0 - Ufx).astype(np.float32)
            bx = 1.0 if k else -1.0
            Kp = Ui + k - LMIN
            out += ax * ay * T[Kp, 0, Jp]
            out += bx * ay * T[Kp, 1, Jp]
            out += ax * by * T[Kp, 2, Jp]
            out += bx * by * T[Kp, 3, Jp]
    return out


def _combine_T(psum):
    """psum [MROWS, NJ, 2, 4] -> T[u][K(NL), m(4), J(NL)]."""
    T = np.zeros((2, NL, 4, NL), np.float32)
    J = np.arange(LMIN, LMIN + NL)
    w0 = 23 - J
    w1 = 24 - J
    for u in range(2):
        p0 = psum[0::2, :, u, :][:, w0, :]  # [t, J, m]
        p1 = psum[1::2, :, u, :][:, w1, :]
        T[u] = (p0 + p1).transpose(0, 2, 1)
    return T


# ----------------------------------------------------------------------------
# device program
# ----------------------------------------------------------------------------

WQ_CHUNKS = (8, 16, 16, 24)      # i-axis chunk sizes (first small: MM0 dep)
SR_CHUNKS = (64, 56, 56)         # y'-axis chunk sizes (chunk0 covers i<8)
N_WARMUP_MM = 26


def _get_nc():
    global _NC
    if _NC is not None:
        return _NC
    sys.path.insert(0, "/opt/trn_rl_repo")
    from contextlib import ExitStack

    import concourse.bass as bass
    import concourse.mybir as mybir
    import concourse.tile as tile
    from concourse import bacc

    nc = bacc.Bacc("TRN2", target_bir_lowering=False, debug=False)
    wq = nc.declare_dram_parameter("wq", [128, 64, 2, MROWS], mybir.dt.float8e4, isOutput=False)
    sra = nc.declare_dram_parameter("sra", [128, 2, SRY, 8], mybir.dt.float8e4, isOutput=False)
    srb = nc.declare_dram_parameter("srb", [128, 2, SRY, 8], mybir.dt.float8e4, isOutput=False)
    tout = nc.declare_dram_parameter("tout", [NGRP, MROWS, NCOLS], mybir.dt.float32, isOutput=True)
    wq_h = wq.tensor if isinstance(wq, bass.AP) else wq
    sra_h = sra.tensor if isinstance(sra, bass.AP) else sra
    srb_h = srb.tensor if isinstance(srb, bass.AP) else srb
    tout_h = tout.tensor if isinstance(tout, bass.AP) else tout

    DR = mybir.MatmulPerfMode.DoubleRow

    with ExitStack() as ctx:
        tc = ctx.enter_context(tile.TileContext(nc))
        pool = ctx.enter_context(tc.tile_pool(name="persist", bufs=1))
        psum_pool = ctx.enter_context(tc.tile_pool(name="psum", bufs=1, space="PSUM"))

        wq_t = pool.tile([128, 64, 2, MROWS], mybir.dt.float8e4)
        sra_t = pool.tile([128, 2, SRY, 8], mybir.dt.float8e4)
        srb_t = pool.tile([128, 2, SRY, 8], mybir.dt.float8e4)

        # PE warm-up: zero-filled dummy DoubleRow matmuls keep the HAM busy
        # while input DMAs land, so the real loop starts at 2.4 GHz.
        wdum = pool.tile([128, 2, 16], mybir.dt.float8e4)
        sdum = pool.tile([128, 2, 128], mybir.dt.float8e4)
        pdum = psum_pool.tile([16, 128], mybir.dt.float32, name="pdum", tag="pdum")
        nc.vector.memset(wdum[:], 0.0)
        nc.vector.memset(sdum[:], 0.0)
        for k in range(N_WARMUP_MM):
            nc.tensor.matmul(pdum[:], wdum[:], sdum[:], start=True, stop=True,
                             perf_mode=DR, skip_group_check=True)

        # chunked input DMAs, split across both HWDGE issuing engines so the
        # ~0.6us per-descriptor issue cost parallelizes; first chunks cover
        # the first matmuls' dependencies so the PE loop starts early.
        i0 = 0
        for csz in WQ_CHUNKS:
            src = bass.AP(tensor=wq_h, offset=i0 * 2 * MROWS,
                          ap=[[64 * 2 * MROWS, 128], [1, csz * 2 * MROWS]])
            nc.sync.dma_start(out=wq_t[:, i0:i0 + csz], in_=src)
            i0 += csz
        y0 = 0
        for csz in SR_CHUNKS:
            y1 = y0 + csz
            for srh, srt in ((sra_h, sra_t), (srb_h, srb_t)):
                src = bass.AP(tensor=srh, offset=y0 * 8,
                              ap=[[2 * SRY * 8, 128], [SRY * 8, 2], [1, (y1 - y0) * 8]])
                nc.scalar.dma_start(out=srt[:, :, y0:y1], in_=src)
            y0 = y1

        psums = [
            psum_pool.tile([MROWS, NCOLS], mybir.dt.float32, name=f"psum{g}", tag=f"psum{g}")
            for g in range(NGRP)
        ]
        for i in range(64):
            lhsT = wq_t[:, i]  # [128, 2, MROWS]
            for g, srt in enumerate((sra_t, srb_t)):
                rhs = srt[:, :, 2 * i:2 * i + NJ, :]  # [128, 2, NJ, 8]
                nc.tensor.matmul(psums[g][:], lhsT, rhs,
                                 start=(i == 0), stop=(i == 63), perf_mode=DR)

        for g in range(NGRP):
            stg = pool.tile([MROWS, NCOLS], mybir.dt.float32, name=f"stg{g}", tag=f"stg{g}")
            nc.vector.tensor_copy(stg[:], psums[g][:])
            dst = bass.AP(tensor=tout_h, offset=g * MROWS * NCOLS,
                          ap=[[NCOLS, MROWS], [1, NCOLS]])
            eng = nc.sync if g == 0 else nc.scalar
            eng.dma_start(out=dst, in_=stg[:])
    nc.compile()
    _NC = nc
    return nc


def _run_device(in_maps, trace=False):
    sys.path.insert(0, "/opt/trn_rl_repo")
    import ml_dtypes
    from concourse.bass_utils import run_bass_kernel_spmd

    fp8 = ml_dtypes.float8_e4m3
    maps = [
        {
            "wq": np.ascontiguousarray(m["wq"]).astype(fp8),
            "sra": np.ascontiguousarray(m["sra"]).astype(fp8),
            "srb": np.ascontiguousarray(m["srb"]).astype(fp8),
        }
        for m in in_maps
    ]
    res = run_bass_kernel_spmd(_get_nc(), maps, core_ids=list(range(len(maps))),
                               trace=trace)
    touts = [
        r["tout"].astype(np.float32).reshape(NGRP, MROWS, NJ, 2, 4) for r in res.results
    ]
    if trace:
        return touts, res
    return touts


# ----------------------------------------------------------------------------
# pipeline
# ----------------------------------------------------------------------------

def _refine_pair_host_only(ego_c, nbr_c, base, align_corners):
    """Pure-host exact fallback (pathological inputs only)."""
    nbrP = _pad_nbr(nbr_c)
    cands = _coarse_cands()
    sc = _exact_scores(ego_c, nbrP, _cand_affines(cands, base), align_corners)
    bi = int(np.argmax(sc))
    cp = cands[bi] if sc[bi] > 1e-5 else np.zeros(3, np.float32)
    if np.all(cp == 0.0):
        return base
    fc = _fine_cands(cp)
    affs_f = _cand_affines(fc, base)
    sf = _exact_scores(ego_c, nbrP, affs_f, align_corners)
    bif = int(np.argmax(sf))
    return affs_f[bif] if sf[bif] > 1e-5 else base


def _finish_pair(ego_c, nbrP, base, cands, approx, align_corners):
    """Adaptive exact rescore of the approx-selected coarse set -> cp."""
    thresh = approx.max() - DELTA_COARSE
    sel = np.where(approx >= thresh)[0]
    if len(sel) > RESCORE_CAP:
        sel = sel[np.argsort(approx[sel])[::-1][:RESCORE_CAP]]
    if len(sel) < 48:
        sel = np.argsort(approx)[::-1][:48]
    affs = _cand_affines(cands[sel], base)
    sc = _exact_scores(ego_c, nbrP, affs, align_corners)
    bi_local = int(np.argmax(sc))
    bi = int(sel[bi_local])
    ok = sc[bi_local] > 1e-5
    cp = cands[bi] if ok else np.zeros(3, np.float32)
    return cp


def kernel(occ_map, record_len, affine_matrix, align_corners):
    occ = np.asarray(occ_map, dtype=np.float32)
    rl = np.asarray(record_len).reshape(-1)
    aff_in = np.asarray(affine_matrix)
    out_dtype = aff_in.dtype
    refined = aff_in.astype(np.float32).copy()
    ac = bool(np.asarray(align_corners))

    # pair list exactly as the reference builds it
    pairs = []
    idx = 0
    for b in range(len(rl)):
        n_agents = int(rl[b])
        grp0 = idx
        idx += n_agents
        if n_agents <= 1:
            continue
        for n in range(1, n_agents):
            pairs.append((b, n, grp0, grp0 + n))
    if not pairs:
        return refined.astype(out_dtype)

    device_ok = (
        len(pairs) <= 2
        and all(
            b < refined.shape[0] and n < refined.shape[2] and nb < occ.shape[0]
            for (b, n, _, nb) in pairs
        )
    )

    pair_data = []
    for (b, n, ei, ni) in pairs:
        # mimic jax OOB semantics: clip gather indices, drop OOB scatters
        ei = min(ei, occ.shape[0] - 1)
        ni = min(ni, occ.shape[0] - 1)
        ego = occ[ei, 0]
        nbr = occ[ni, 0]
        ego_c = np.where(ego > THRESH, ego, 0.0).astype(np.float32)
        nbr_c = np.where(nbr > THRESH, nbr, 0.0).astype(np.float32)
        base = refined[b, 0, n].astype(np.float32)
        pair_data.append(
            {
                "b": min(b, refined.shape[0] - 1),
                "n": n,
                "ego_c": ego_c,
                "nbr_c": nbr_c,
                "nbrP": _pad_nbr(nbr_c),
                "base": base,
            }
        )

    cands = _coarse_cands()
    drs = np.unique(cands[:, 2])  # 16 rotations
    by_dr = {float(dr): np.where(cands[:, 2] == dr)[0] for dr in drs}

    # build device inputs: 16 theta-units per pair, 4 per core; cores 0-3 pair0,
    # cores 4-7 pair1
    use_device = device_ok
    unit_map = {}  # (core, u) -> (pair_idx, dr)
    in_maps = None
    if use_device:
        zero_wq = np.zeros((128, 64, 2, MROWS), np.float32)
        zero_sr = np.zeros((128, 2, SRY, 8), np.float32)
        in_maps = []
        splat_fail = False
        wq_cache = {}
        for core in range(N_CORES):
            pi = core // 4
            if pi >= len(pair_data):
                in_maps.append({"wq": zero_wq, "sra": zero_sr, "srb": zero_sr})
                continue
            pd = pair_data[pi]
            if pi not in wq_cache:
                wq_cache[pi] = _build_wq(pd["nbr_c"])
            srs = []
            for g in range(NGRP):
                SR = np.zeros((128, 2, SRY, 8), np.float32)
                for ug in range(2):
                    u = 2 * g + ug
                    th_idx = 4 * (core % 4) + u
                    dr = float(drs[th_idx])
                    ix, iy = _theta_warp_fields(pd["base"], dr, ac)
                    S = _build_splats(pd["ego_c"], ix, iy)
                    if S is None:
                        splat_fail = True
                        break
                    # SR[c, h, y', u, m] = S[m, SRY0 + y', c + 128h]
                    blk = S[:, SRY0:SRY0 + SRY, :].transpose(2, 1, 0)  # [cx, y', m]
                    SR[:, :, :, 4 * ug:4 * ug + 4] = (
                        blk.reshape(2, 128, SRY, 4).transpose(1, 0, 2, 3)
                    )
                    unit_map[(core, u)] = (pi, dr)
                if splat_fail:
                    break
                srs.append(SR)
            if splat_fail:
                break
            in_maps.append({"wq": wq_cache[pi], "sra": srs[0], "srb": srs[1]})
        if splat_fail:
            use_device = False

    if use_device:
        try:
            global _LAST_IN_MAPS
            _LAST_IN_MAPS = in_maps
            touts = _run_device(in_maps)
        except Exception:
            use_device = False

    for pi, pd in enumerate(pair_data):
        base = pd["base"]
        pair_device = use_device
        approx = None
        if pair_device:
            approx = np.empty(len(cands), np.float32)
            for core in range(4 * pi, 4 * pi + 4):
                for g in range(NGRP):
                    Tg = _combine_T(touts[core][g])
                    for ug in range(2):
                        u = 2 * g + ug
                        key = (core, u)
                        if key not in unit_map:
                            continue
                        _, dr = unit_map[key]
                        sel = by_dr[dr]
                        a = _assemble_approx(Tg[ug], base, cands[sel], ac)
                        if a is None:
                            pair_device = False
                            break
                        approx[sel] = a
                    if not pair_device:
                        break
                if not pair_device:
                    break
        if pair_device:
            cp = _finish_pair(pd["ego_c"], pd["nbrP"], base, cands, approx, ac)
            if np.all(cp == 0.0):
                new_aff = base
            else:
                fc = _fine_cands(cp)
                affs_f = _cand_affines(fc, base)
                sf = _exact_scores(pd["ego_c"], pd["nbrP"], affs_f, ac)
                bif = int(np.argmax(sf))
                new_aff = affs_f[bif] if sf[bif] > 1e-5 else base
        else:
            new_aff = _refine_pair_host_only(pd["ego_c"], pd["nbr_c"], base, ac)
        if pd["n"] < refined.shape[2] and pd["b"] < refined.shape[0]:
            refined[pd["b"], 0, pd["n"]] = new_aff

    return refined.astype(out_dtype)


# revision 10
# speedup vs baseline: 1.0620x; 1.0133x over previous
"""Trainium2 Bass kernel for nn_CartographerPoseCorrector.

Strategy
--------
The reference refines, per (ego, nbr) pair, a 2x3 affine by scoring 7056
coarse + 729 fine candidate warps (bilinear grid-sample of nbr against ego)
and picking the argmax of each stage.

Device (8 NeuronCores, SPMD): for every coarse rotation theta (16 per pair,
sharded 4 per core; pairs split across core halves) compute integer-lag
moment-correlation surfaces on the TensorEngine:

    T_m[K,J] = sum_p mu_m(p) * ego[p] * nbr~[Yi(p)+J, Xi(p)+K]

for mu_m in {1, Xf, Yf, Xf*Yf}, lags J,K in [-24, 24).  (Yi,Xi / Yf,Xf are
the integer/fractional parts of the theta-warp sample positions; the
candidate-translation axis of the search grid collapses onto the lag axes.)

The device program uses fp8e4 DoubleRow matmuls: contraction = 256 (canvas-X
split into the Ko=2 interleave), 64 accumulation steps over image-row pairs,
two 2-unit matmul groups per step.  All operand layouts are prebuilt on the
host so the kernel is DMA -> 128 matmuls -> DMA out.

From these surfaces the host assembles, per candidate, the exact
no-carry-bilinear approximation of its score, keeps every candidate within a
safety margin of the max, exactly rescores that small set (and the 729 fine
candidates) in fp32, and takes the argmax - reproducing the reference's
selection exactly.  A tiny host argmax/gather finishes, per the sharding
hint.
"""

import math
import sys

import numpy as np

H = W = 128
THRESH = 0.3
TRANS_RANGE = 20.0
ROT_RANGE = 15.0
COARSE_STEP = 2.0
FINE_STEP = 0.5

# Device-kernel geometry (must match the Bass program)
NL = 48          # lags per axis
LMIN = -24       # lag range [LMIN, LMIN + NL)
NJ = NL + 1      # sliding J-window width
MROWS = 2 * NL   # psum rows: (t, slot)
OFFX = OFFY = 44 # image coord -> canvas coord offset
CY = 224         # canvas Y extent
CX = 256         # canvas X extent (2 Ko chunks of 128)
SRY = 176        # stored canvas-Y rows (window only touches y in [21, 196))
SRY0 = 21        # stored rows are S[.., SRY0 + y', ..]
U = 4            # units (theta-warps) per core
NGRP = 2         # matmul groups (2 units each)
NCOLS = NJ * 2 * 4  # 392 psum cols: (window pos, unit-in-group, moment)
N_CORES = 8

DELTA_COARSE = 280.0   # exact-rescore safety margin (measured errmax ~97)
RESCORE_CAP = 2200     # hard cap on rescored coarse candidates per pair

_NC = None


# ----------------------------------------------------------------------------
# host math (mirrors reference numerics in fp32 where it matters)
# ----------------------------------------------------------------------------

def _grid_1d(align_corners):
    if align_corners:
        xs = np.linspace(-1.0, 1.0, W, dtype=np.float32)
        ys = np.linspace(-1.0, 1.0, H, dtype=np.float32)
    else:
        xs = ((2.0 * np.arange(W, dtype=np.float32) + 1.0) / W - 1.0)
        ys = ((2.0 * np.arange(H, dtype=np.float32) + 1.0) / H - 1.0)
    return xs, ys


def _coarse_cands():
    dxs = np.arange(-TRANS_RANGE, TRANS_RANGE + 1e-3, COARSE_STEP, dtype=np.float32)
    drs = np.arange(-ROT_RANGE, ROT_RANGE + 1e-3, COARSE_STEP, dtype=np.float32)
    gdx, gdy, gdr = np.meshgrid(dxs, dxs, drs, indexing="ij")
    return np.stack([gdx.ravel(), gdy.ravel(), gdr.ravel()], axis=1)


def _fine_cands(cp):
    off = np.arange(-COARSE_STEP, COARSE_STEP + 1e-3, FINE_STEP, dtype=np.float32)
    gdx, gdy, gdr = np.meshgrid(cp[0] + off, cp[1] + off, cp[2] + off, indexing="ij")
    return np.stack([gdx.ravel(), gdy.ravel(), gdr.ravel()], axis=1)


def _cand_affines(cands, base_2x3):
    dx, dy, dr = cands[:, 0], cands[:, 1], cands[:, 2]
    tx = (2.0 * dx / max(W - 1, 1)).astype(np.float32)
    ty = (2.0 * dy / max(H - 1, 1)).astype(np.float32)
    th = (dr * np.float32(math.pi / 180.0)).astype(np.float32)
    c, s = np.cos(th), np.sin(th)
    z, o = np.zeros_like(c), np.ones_like(c)
    delta = np.stack([c, -s, tx, s, c, ty, z, z, o], axis=-1).reshape(-1, 3, 3)
    base3 = np.concatenate([base_2x3, np.array([[0, 0, 1]], np.float32)], axis=0)
    return np.einsum("ij,njk->nik", base3.astype(np.float32), delta.astype(np.float32))[
        :, :2, :
    ].astype(np.float32)


def _pad_nbr(nbr_c, padb=8):
    out = np.zeros((H + 2 * padb, W + 2 * padb), np.float32)
    out[padb : padb + H, padb : padb + W] = nbr_c
    return out


def _exact_scores(ego_c, nbrP, affs, align_corners, padb=8, chunk=16):
    """Exact fp32 bilinear grid-sample scores for candidate affines [n,2,3]."""
    xs, ys = _grid_1d(align_corners)
    gx = np.broadcast_to(xs[None, :], (H, W)).ravel().astype(np.float32)
    gy = np.broadcast_to(ys[:, None], (H, W)).ravel().astype(np.float32)
    flat = nbrP.ravel()
    Wp = nbrP.shape[1]
    if align_corners:
        scx, ox = np.float32(0.5 * (W - 1)), np.float32(0.5 * (W - 1))
        scy, oy = np.float32(0.5 * (H - 1)), np.float32(0.5 * (H - 1))
    else:
        scx, ox = np.float32(0.5 * W), np.float32(0.5 * W - 0.5)
        scy, oy = np.float32(0.5 * H), np.float32(0.5 * H - 0.5)
    ego_f = ego_c.ravel().astype(np.float32)
    N = len(affs)
    out = np.empty(N, np.float32)
    for s0 in range(0, N, chunk):
        A = affs[s0 : s0 + chunk].astype(np.float32)
        n = len(A)
        ix = np.multiply.outer(A[:, 0, 0], gx)
        ix += np.multiply.outer(A[:, 0, 1], gy)
        ix += A[:, 0, 2, None]
        ix *= scx
        ix += ox
        iy = np.multiply.outer(A[:, 1, 0], gx)
        iy += np.multiply.outer(A[:, 1, 1], gy)
        iy += A[:, 1, 2, None]
        iy *= scy
        iy += oy
        x0 = np.floor(ix)
        y0 = np.floor(iy)
        wx = ix - x0
        wy = iy - y0
        xi = x0.astype(np.int32)
        xi += padb
        np.clip(xi, 0, Wp - 2, out=xi)
        yi = y0.astype(np.int32)
        yi += padb
        np.clip(yi, 0, Wp - 2, out=yi)
        base = yi
        base *= Wp
        base += xi
        b00 = flat[base]
        b01 = flat[base + 1]
        b10 = flat[base + Wp]
        b11 = flat[base + Wp + 1]
        top = (1.0 - wx) * b00
        top += wx * b01
        bot = (1.0 - wx) * b10
        bot += wx * b11
        val = (1.0 - wy) * top
        val += wy * bot
        out[s0 : s0 + n] = val @ ego_f
    return out


def _theta_warp_fields(base_2x3, dr, align_corners):
    """Pixel-coord sample positions of the theta-only warp (dx=dy=0)."""
    th = np.float32(dr) * np.float32(math.pi / 180.0)
    c, s = np.cos(th, dtype=np.float32), np.sin(th, dtype=np.float32)
    delta = np.array([[c, -s, 0], [s, c, 0], [0, 0, 1]], np.float32)
    base3 = np.concatenate([base_2x3, [[0, 0, 1]]], 0).astype(np.float32)
    aff = (base3 @ delta)[:2]
    xs, ys = _grid_1d(align_corners)
    gx = aff[0, 0] * xs[None, :] + aff[0, 1] * ys[:, None] + aff[0, 2]
    gy = aff[1, 0] * xs[None, :] + aff[1, 1] * ys[:, None] + aff[1, 2]
    if align_corners:
        ix = (gx + 1.0) * (0.5 * (W - 1))
        iy = (gy + 1.0) * (0.5 * (H - 1))
    else:
        ix = gx * (0.5 * W) + (0.5 * W - 0.5)
        iy = gy * (0.5 * H) + (0.5 * H - 0.5)
    return ix.astype(np.float32), iy.astype(np.float32)


def _trans_shifts(base_2x3, cands, align_corners):
    """Pixel-space shifts (ux, uy) each candidate translation adds."""
    B2 = base_2x3[:2, :2].astype(np.float32)
    tx = (2.0 * cands[:, 0] / (W - 1)).astype(np.float32)
    ty = (2.0 * cands[:, 1] / (H - 1)).astype(np.float32)
    if align_corners:
        sx, sy = 0.5 * (W - 1), 0.5 * (H - 1)
    else:
        sx, sy = 0.5 * W, 0.5 * H
    ux = (B2[0, 0] * tx + B2[0, 1] * ty) * np.float32(sx)
    uy = (B2[1, 0] * tx + B2[1, 1] * ty) * np.float32(sy)
    return ux, uy


def _build_splats(ego_c, ix, iy):
    """Moment splat canvases [4, CY, CX] f32, or None if out of range."""
    Xi = np.floor(ix)
    Yi = np.floor(iy)
    Xf = (ix - Xi).astype(np.float32)
    Yf = (iy - Yi).astype(np.float32)
    Xi = Xi.astype(np.int64)
    Yi = Yi.astype(np.int64)
    if (
        Xi.min() < -OFFX
        or Xi.max() >= CX - OFFX
        or Yi.min() < -OFFY
        or Yi.max() >= CY - OFFY
    ):
        return None
    S = np.zeros((4, CY, CX), np.float32)
    flatidx = ((Yi + OFFY) * CX + (Xi + OFFX)).ravel()
    nbins = CY * CX
    for m, mu in enumerate((None, Xf, Yf, Xf * Yf)):
        wgt = ego_c if mu is None else mu * ego_c
        S[m] = (
            np.bincount(flatidx, weights=wgt.ravel().astype(np.float64), minlength=nbins)
            .reshape(CY, CX)
            .astype(np.float32)
        )
    return S


def _build_wq(nbr_c):
    """Stationary windows WQ[c, i, h, 2t+slot] = nbr[2i+slot, c+128h+t-68]."""
    WQ = np.zeros((128, 64, 2, MROWS), np.float32)
    c = np.arange(128)[:, None, None]
    h = np.arange(2)[None, :, None]
    t = np.arange(NL)[None, None, :]
    v = c + 128 * h + t - 68
    valid = (v >= 0) & (v < W)
    vc = np.clip(v, 0, W - 1)
    for slot in range(2):
        # [c, h, t] gather per row y -> place at [c, i, h, slot::2]
        rows = nbr_c[slot::2, :]  # [64, W]
        vals = np.where(valid[None], rows[:, vc], 0.0)  # [64, c, h, t]
        WQ[:, :, :, slot::2] = vals.transpose(1, 0, 2, 3)
    return WQ


def _assemble_approx(T, base_2x3, cands, align_corners):
    """Approx scores for one theta's candidates from its surface T [NL, 4, NL].

    Returns None if any candidate's lag falls outside the computed window
    (caller falls back to the exact host path)."""
    ux, uy = _trans_shifts(base_2x3, cands, align_corners)
    Ui = np.floor(ux).astype(np.int64)
    Ufx = (ux - Ui).astype(np.float32)
    Vi = np.floor(uy).astype(np.int64)
    Ufy = (uy - Vi).astype(np.float32)
    if (
        Ui.min() < LMIN
        or Ui.max() + 1 >= LMIN + NL
        or Vi.min() < LMIN
        or Vi.max() + 1 >= LMIN + NL
    ):
        return None
    out = np.zeros(len(cands), np.float32)
    for j in (0, 1):
        ay = np.where(j, Ufy, 1.0 - Ufy).astype(np.float32)
        by = 1.0 if j else -1.0
        Jp = Vi + j - LMIN
        for k in (0, 1):
            ax = np.where(k, Ufx, 1.0 - Ufx).astype(np.float32)
            bx = 1.0 if k else -1.0
            Kp = Ui + k - LMIN
            out += ax * ay * T[Kp, 0, Jp]
            out += bx * ay * T[Kp, 1, Jp]
            out += ax * by * T[Kp, 2, Jp]
            out += bx * by * T[Kp, 3, Jp]
    return out


def _combine_T(psum):
    """psum [MROWS, NJ, 2, 4] -> T[u][K(NL), m(4), J(NL)]."""
    T = np.zeros((2, NL, 4, NL), np.float32)
    J = np.arange(LMIN, LMIN + NL)
    w0 = 23 - J
    w1 = 24 - J
    for u in range(2):
        p0 = psum[0::2, :, u, :][:, w0, :]  # [t, J, m]
        p1 = psum[1::2, :, u, :][:, w1, :]
        T[u] = (p0 + p1).transpose(0, 2, 1)
    return T


# ----------------------------------------------------------------------------
# device program
# ----------------------------------------------------------------------------

WQ_CHUNKS = (8, 16, 16, 24)      # i-axis chunk sizes (first small: MM0 dep)
SR_CHUNKS = (64, 56, 56)         # y'-axis chunk sizes (chunk0 covers i<8)
N_WARMUP_MM = 12                 # 512-col dummies: ~4.3us of PE busy to warm HAM


def _get_nc():
    global _NC
    if _NC is not None:
        return _NC
    sys.path.insert(0, "/opt/trn_rl_repo")
    from contextlib import ExitStack

    import concourse.bass as bass
    import concourse.mybir as mybir
    import concourse.tile as tile
    from concourse import bacc

    nc = bacc.Bacc("TRN2", target_bir_lowering=False, debug=False)
    wq = nc.declare_dram_parameter("wq", [128, 64, 2, MROWS], mybir.dt.float8e4, isOutput=False)
    sra = nc.declare_dram_parameter("sra", [128, 2, SRY, 8], mybir.dt.float8e4, isOutput=False)
    srb = nc.declare_dram_parameter("srb", [128, 2, SRY, 8], mybir.dt.float8e4, isOutput=False)
    tout = nc.declare_dram_parameter("tout", [NGRP, MROWS, NCOLS], mybir.dt.float32, isOutput=True)
    wq_h = wq.tensor if isinstance(wq, bass.AP) else wq
    sra_h = sra.tensor if isinstance(sra, bass.AP) else sra
    srb_h = srb.tensor if isinstance(srb, bass.AP) else srb
    tout_h = tout.tensor if isinstance(tout, bass.AP) else tout

    DR = mybir.MatmulPerfMode.DoubleRow

    with ExitStack() as ctx:
        tc = ctx.enter_context(tile.TileContext(nc))
        pool = ctx.enter_context(tc.tile_pool(name="persist", bufs=1))
        psum_pool = ctx.enter_context(tc.tile_pool(name="psum", bufs=1, space="PSUM"))

        wq_t = pool.tile([128, 64, 2, MROWS], mybir.dt.float8e4)
        sra_t = pool.tile([128, 2, SRY, 8], mybir.dt.float8e4)
        srb_t = pool.tile([128, 2, SRY, 8], mybir.dt.float8e4)

        # PE warm-up: zero-filled dummy DoubleRow matmuls keep the HAM busy
        # while input DMAs land, so the real loop starts at 2.4 GHz.
        wdum = pool.tile([128, 2, 16], mybir.dt.float8e4)
        sdum = pool.tile([128, 2, 512], mybir.dt.float8e4)
        pdum = psum_pool.tile([16, 512], mybir.dt.float32, name="pdum", tag="pdum")
        nc.vector.memset(wdum[:], 0.0)
        nc.vector.memset(sdum[:], 0.0)
        for k in range(N_WARMUP_MM):
            nc.tensor.matmul(pdum[:], wdum[:], sdum[:], start=True, stop=True,
                             perf_mode=DR, skip_group_check=True)

        # chunked input DMAs, split across both HWDGE issuing engines so the
        # ~0.6us per-descriptor issue cost parallelizes; each engine's first
        # descriptor covers MM0's dependencies so the PE loop starts early.
        def wq_src(i0, csz):
            return bass.AP(tensor=wq_h, offset=i0 * 2 * MROWS,
                           ap=[[64 * 2 * MROWS, 128], [1, csz * 2 * MROWS]])

        def sr_src(srh, y0, y1):
            return bass.AP(tensor=srh, offset=y0 * 8,
                           ap=[[2 * SRY * 8, 128], [SRY * 8, 2], [1, (y1 - y0) * 8]])

        wq_offs = [sum(WQ_CHUNKS[:k]) for k in range(len(WQ_CHUNKS))]
        sr_offs = [sum(SR_CHUNKS[:k]) for k in range(len(SR_CHUNKS))]
        sync_plan = [("wq", 0), ("srb", 0), ("wq", 1), ("wq", 2), ("wq", 3)]
        scal_plan = [("sra", 0), ("sra", 1), ("srb", 1), ("sra", 2), ("srb", 2)]
        for eng, plan in ((nc.sync, sync_plan), (nc.scalar, scal_plan)):
            for kind, k in plan:
                if kind == "wq":
                    eng.dma_start(out=wq_t[:, wq_offs[k]:wq_offs[k] + WQ_CHUNKS[k]],
                                  in_=wq_src(wq_offs[k], WQ_CHUNKS[k]))
                else:
                    srh, srt = (sra_h, sra_t) if kind == "sra" else (srb_h, srb_t)
                    y0, y1 = sr_offs[k], sr_offs[k] + SR_CHUNKS[k]
                    eng.dma_start(out=srt[:, :, y0:y1], in_=sr_src(srh, y0, y1))

        psums = [
            psum_pool.tile([MROWS, NCOLS], mybir.dt.float32, name=f"psum{g}", tag=f"psum{g}")
            for g in range(NGRP)
        ]
        for i in range(64):
            lhsT = wq_t[:, i]  # [128, 2, MROWS]
            for g, srt in enumerate((sra_t, srb_t)):
                rhs = srt[:, :, 2 * i:2 * i + NJ, :]  # [128, 2, NJ, 8]
                nc.tensor.matmul(psums[g][:], lhsT, rhs,
                                 start=(i == 0), stop=(i == 63), perf_mode=DR)

        for g in range(NGRP):
            stg = pool.tile([MROWS, NCOLS], mybir.dt.float32, name=f"stg{g}", tag=f"stg{g}")
            nc.vector.tensor_copy(stg[:], psums[g][:])
            dst = bass.AP(tensor=tout_h, offset=g * MROWS * NCOLS,
                          ap=[[NCOLS, MROWS], [1, NCOLS]])
            eng = nc.sync if g == 0 else nc.scalar
            eng.dma_start(out=dst, in_=stg[:])
    nc.compile()
    _NC = nc
    return nc


def _run_device(in_maps, trace=False):
    sys.path.insert(0, "/opt/trn_rl_repo")
    import ml_dtypes
    from concourse.bass_utils import run_bass_kernel_spmd

    fp8 = ml_dtypes.float8_e4m3
    maps = [
        {
            "wq": np.ascontiguousarray(m["wq"]).astype(fp8),
            "sra": np.ascontiguousarray(m["sra"]).astype(fp8),
            "srb": np.ascontiguousarray(m["srb"]).astype(fp8),
        }
        for m in in_maps
    ]
    res = run_bass_kernel_spmd(_get_nc(), maps, core_ids=list(range(len(maps))),
                               trace=trace)
    touts = [
        r["tout"].astype(np.float32).reshape(NGRP, MROWS, NJ, 2, 4) for r in res.results
    ]
    if trace:
        return touts, res
    return touts


# ----------------------------------------------------------------------------
# pipeline
# ----------------------------------------------------------------------------

def _refine_pair_host_only(ego_c, nbr_c, base, align_corners):
    """Pure-host exact fallback (pathological inputs only)."""
    nbrP = _pad_nbr(nbr_c)
    cands = _coarse_cands()
    sc = _exact_scores(ego_c, nbrP, _cand_affines(cands, base), align_corners)
    bi = int(np.argmax(sc))
    cp = cands[bi] if sc[bi] > 1e-5 else np.zeros(3, np.float32)
    if np.all(cp == 0.0):
        return base
    fc = _fine_cands(cp)
    affs_f = _cand_affines(fc, base)
    sf = _exact_scores(ego_c, nbrP, affs_f, align_corners)
    bif = int(np.argmax(sf))
    return affs_f[bif] if sf[bif] > 1e-5 else base


def _finish_pair(ego_c, nbrP, base, cands, approx, align_corners):
    """Adaptive exact rescore of the approx-selected coarse set -> cp."""
    thresh = approx.max() - DELTA_COARSE
    sel = np.where(approx >= thresh)[0]
    if len(sel) > RESCORE_CAP:
        sel = sel[np.argsort(approx[sel])[::-1][:RESCORE_CAP]]
    if len(sel) < 48:
        sel = np.argsort(approx)[::-1][:48]
    affs = _cand_affines(cands[sel], base)
    sc = _exact_scores(ego_c, nbrP, affs, align_corners)
    bi_local = int(np.argmax(sc))
    bi = int(sel[bi_local])
    ok = sc[bi_local] > 1e-5
    cp = cands[bi] if ok else np.zeros(3, np.float32)
    return cp


def kernel(occ_map, record_len, affine_matrix, align_corners):
    occ = np.asarray(occ_map, dtype=np.float32)
    rl = np.asarray(record_len).reshape(-1)
    aff_in = np.asarray(affine_matrix)
    out_dtype = aff_in.dtype
    refined = aff_in.astype(np.float32).copy()
    ac = bool(np.asarray(align_corners))

    # pair list exactly as the reference builds it
    pairs = []
    idx = 0
    for b in range(len(rl)):
        n_agents = int(rl[b])
        grp0 = idx
        idx += n_agents
        if n_agents <= 1:
            continue
        for n in range(1, n_agents):
            pairs.append((b, n, grp0, grp0 + n))
    if not pairs:
        return refined.astype(out_dtype)

    device_ok = (
        len(pairs) <= 2
        and all(
            b < refined.shape[0] and n < refined.shape[2] and nb < occ.shape[0]
            for (b, n, _, nb) in pairs
        )
    )

    pair_data = []
    for (b, n, ei, ni) in pairs:
        # mimic jax OOB semantics: clip gather indices, drop OOB scatters
        ei = min(ei, occ.shape[0] - 1)
        ni = min(ni, occ.shape[0] - 1)
        ego = occ[ei, 0]
        nbr = occ[ni, 0]
        ego_c = np.where(ego > THRESH, ego, 0.0).astype(np.float32)
        nbr_c = np.where(nbr > THRESH, nbr, 0.0).astype(np.float32)
        base = refined[b, 0, n].astype(np.float32)
        pair_data.append(
            {
                "b": min(b, refined.shape[0] - 1),
                "n": n,
                "ego_c": ego_c,
                "nbr_c": nbr_c,
                "nbrP": _pad_nbr(nbr_c),
                "base": base,
            }
        )

    cands = _coarse_cands()
    drs = np.unique(cands[:, 2])  # 16 rotations
    by_dr = {float(dr): np.where(cands[:, 2] == dr)[0] for dr in drs}

    # build device inputs: 16 theta-units per pair, 4 per core; cores 0-3 pair0,
    # cores 4-7 pair1
    use_device = device_ok
    unit_map = {}  # (core, u) -> (pair_idx, dr)
    in_maps = None
    if use_device:
        zero_wq = np.zeros((128, 64, 2, MROWS), np.float32)
        zero_sr = np.zeros((128, 2, SRY, 8), np.float32)
        in_maps = []
        splat_fail = False
        wq_cache = {}
        for core in range(N_CORES):
            pi = core // 4
            if pi >= len(pair_data):
                in_maps.append({"wq": zero_wq, "sra": zero_sr, "srb": zero_sr})
                continue
            pd = pair_data[pi]
            if pi not in wq_cache:
                wq_cache[pi] = _build_wq(pd["nbr_c"])
            srs = []
            for g in range(NGRP):
                SR = np.zeros((128, 2, SRY, 8), np.float32)
                for ug in range(2):
                    u = 2 * g + ug
                    th_idx = 4 * (core % 4) + u
                    dr = float(drs[th_idx])
                    ix, iy = _theta_warp_fields(pd["base"], dr, ac)
                    S = _build_splats(pd["ego_c"], ix, iy)
                    if S is None:
                        splat_fail = True
                        break
                    # SR[c, h, y', u, m] = S[m, SRY0 + y', c + 128h]
                    blk = S[:, SRY0:SRY0 + SRY, :].transpose(2, 1, 0)  # [cx, y', m]
                    SR[:, :, :, 4 * ug:4 * ug + 4] = (
                        blk.reshape(2, 128, SRY, 4).transpose(1, 0, 2, 3)
                    )
                    unit_map[(core, u)] = (pi, dr)
                if splat_fail:
                    break
                srs.append(SR)
            if splat_fail:
                break
            in_maps.append({"wq": wq_cache[pi], "sra": srs[0], "srb": srs[1]})
        if splat_fail:
            use_device = False

    if use_device:
        try:
            global _LAST_IN_MAPS
            _LAST_IN_MAPS = in_maps
            touts = _run_device(in_maps)
        except Exception:
            use_device = False

    for pi, pd in enumerate(pair_data):
        base = pd["base"]
        pair_device = use_device
        approx = None
        if pair_device:
            approx = np.empty(len(cands), np.float32)
            for core in range(4 * pi, 4 * pi + 4):
                for g in range(NGRP):
                    Tg = _combine_T(touts[core][g])
                    for ug in range(2):
                        u = 2 * g + ug
                        key = (core, u)
                        if key not in unit_map:
                            continue
                        _, dr = unit_map[key]
                        sel = by_dr[dr]
                        a = _assemble_approx(Tg[ug], base, cands[sel], ac)
                        if a is None:
                            pair_device = False
                            break
                        approx[sel] = a
                    if not pair_device:
                        break
                if not pair_device:
                    break
        if pair_device:
            cp = _finish_pair(pd["ego_c"], pd["nbrP"], base, cands, approx, ac)
            if np.all(cp == 0.0):
                new_aff = base
            else:
                fc = _fine_cands(cp)
                affs_f = _cand_affines(fc, base)
                sf = _exact_scores(pd["ego_c"], pd["nbrP"], affs_f, ac)
                bif = int(np.argmax(sf))
                new_aff = affs_f[bif] if sf[bif] > 1e-5 else base
        else:
            new_aff = _refine_pair_host_only(pd["ego_c"], pd["nbr_c"], base, ac)
        if pd["n"] < refined.shape[2] and pd["b"] < refined.shape[0]:
            refined[pd["b"], 0, pd["n"]] = new_aff

    return refined.astype(out_dtype)


# revision 11
# speedup vs baseline: 1.1019x; 1.0375x over previous
"""Trainium2 Bass kernel for nn_CartographerPoseCorrector.

Strategy
--------
The reference refines, per (ego, nbr) pair, a 2x3 affine by scoring 7056
coarse + 729 fine candidate warps (bilinear grid-sample of nbr against ego)
and picking the argmax of each stage.

Device (8 NeuronCores, SPMD): for every coarse rotation theta (16 per pair,
sharded 4 per core; pairs split across core halves) compute integer-lag
moment-correlation surfaces on the TensorEngine:

    T_m[K,J] = sum_p mu_m(p) * ego[p] * nbr~[Yi(p)+J, Xi(p)+K]

for mu_m in {1, Xf, Yf, Xf*Yf}, lags J,K in [-24, 24).  (Yi,Xi / Yf,Xf are
the integer/fractional parts of the theta-warp sample positions; the
candidate-translation axis of the search grid collapses onto the lag axes.)

The device program uses fp8e4 DoubleRow matmuls: contraction = 256 (canvas-X
split into the Ko=2 interleave), 64 accumulation steps over image-row pairs,
two 2-unit matmul groups per step.  All operand layouts are prebuilt on the
host so the kernel is DMA -> 128 matmuls -> DMA out.

From these surfaces the host assembles, per candidate, the exact
no-carry-bilinear approximation of its score, keeps every candidate within a
safety margin of the max, exactly rescores that small set (and the 729 fine
candidates) in fp32, and takes the argmax - reproducing the reference's
selection exactly.  A tiny host argmax/gather finishes, per the sharding
hint.
"""

import math
import sys

import numpy as np

H = W = 128
THRESH = 0.3
TRANS_RANGE = 20.0
ROT_RANGE = 15.0
COARSE_STEP = 2.0
FINE_STEP = 0.5

# Device-kernel geometry (must match the Bass program)
NL = 48          # lags per axis
LMIN = -24       # lag range [LMIN, LMIN + NL)
NJ = NL + 1      # sliding J-window width
MROWS = 2 * NL   # psum rows: (t, slot)
OFFX = OFFY = 44 # image coord -> canvas coord offset
CY = 224         # canvas Y extent
CX = 256         # canvas X extent (2 Ko chunks of 128)
SRY = 176        # stored canvas-Y rows (window only touches y in [21, 196))
SRY0 = 21        # stored rows are S[.., SRY0 + y', ..]
U = 4            # units (theta-warps) per core
NGRP = 2         # matmul groups (2 units each)
NCOLS = NJ * 2 * 4  # 392 psum cols: (window pos, unit-in-group, moment)
N_CORES = 8

DELTA_COARSE = 280.0   # exact-rescore safety margin (measured errmax ~97)
RESCORE_CAP = 2200     # hard cap on rescored coarse candidates per pair

_NC = None


# ----------------------------------------------------------------------------
# host math (mirrors reference numerics in fp32 where it matters)
# ----------------------------------------------------------------------------

def _grid_1d(align_corners):
    if align_corners:
        xs = np.linspace(-1.0, 1.0, W, dtype=np.float32)
        ys = np.linspace(-1.0, 1.0, H, dtype=np.float32)
    else:
        xs = ((2.0 * np.arange(W, dtype=np.float32) + 1.0) / W - 1.0)
        ys = ((2.0 * np.arange(H, dtype=np.float32) + 1.0) / H - 1.0)
    return xs, ys


def _coarse_cands():
    dxs = np.arange(-TRANS_RANGE, TRANS_RANGE + 1e-3, COARSE_STEP, dtype=np.float32)
    drs = np.arange(-ROT_RANGE, ROT_RANGE + 1e-3, COARSE_STEP, dtype=np.float32)
    gdx, gdy, gdr = np.meshgrid(dxs, dxs, drs, indexing="ij")
    return np.stack([gdx.ravel(), gdy.ravel(), gdr.ravel()], axis=1)


def _fine_cands(cp):
    off = np.arange(-COARSE_STEP, COARSE_STEP + 1e-3, FINE_STEP, dtype=np.float32)
    gdx, gdy, gdr = np.meshgrid(cp[0] + off, cp[1] + off, cp[2] + off, indexing="ij")
    return np.stack([gdx.ravel(), gdy.ravel(), gdr.ravel()], axis=1)


def _cand_affines(cands, base_2x3):
    dx, dy, dr = cands[:, 0], cands[:, 1], cands[:, 2]
    tx = (2.0 * dx / max(W - 1, 1)).astype(np.float32)
    ty = (2.0 * dy / max(H - 1, 1)).astype(np.float32)
    th = (dr * np.float32(math.pi / 180.0)).astype(np.float32)
    c, s = np.cos(th), np.sin(th)
    z, o = np.zeros_like(c), np.ones_like(c)
    delta = np.stack([c, -s, tx, s, c, ty, z, z, o], axis=-1).reshape(-1, 3, 3)
    base3 = np.concatenate([base_2x3, np.array([[0, 0, 1]], np.float32)], axis=0)
    return np.einsum("ij,njk->nik", base3.astype(np.float32), delta.astype(np.float32))[
        :, :2, :
    ].astype(np.float32)


def _pad_nbr(nbr_c, padb=8):
    out = np.zeros((H + 2 * padb, W + 2 * padb), np.float32)
    out[padb : padb + H, padb : padb + W] = nbr_c
    return out


def _exact_scores(ego_c, nbrP, affs, align_corners, padb=8, chunk=16):
    """Exact fp32 bilinear grid-sample scores for candidate affines [n,2,3]."""
    xs, ys = _grid_1d(align_corners)
    gx = np.broadcast_to(xs[None, :], (H, W)).ravel().astype(np.float32)
    gy = np.broadcast_to(ys[:, None], (H, W)).ravel().astype(np.float32)
    flat = nbrP.ravel()
    Wp = nbrP.shape[1]
    if align_corners:
        scx, ox = np.float32(0.5 * (W - 1)), np.float32(0.5 * (W - 1))
        scy, oy = np.float32(0.5 * (H - 1)), np.float32(0.5 * (H - 1))
    else:
        scx, ox = np.float32(0.5 * W), np.float32(0.5 * W - 0.5)
        scy, oy = np.float32(0.5 * H), np.float32(0.5 * H - 0.5)
    ego_f = ego_c.ravel().astype(np.float32)
    N = len(affs)
    out = np.empty(N, np.float32)
    for s0 in range(0, N, chunk):
        A = affs[s0 : s0 + chunk].astype(np.float32)
        n = len(A)
        ix = np.multiply.outer(A[:, 0, 0], gx)
        ix += np.multiply.outer(A[:, 0, 1], gy)
        ix += A[:, 0, 2, None]
        ix *= scx
        ix += ox
        iy = np.multiply.outer(A[:, 1, 0], gx)
        iy += np.multiply.outer(A[:, 1, 1], gy)
        iy += A[:, 1, 2, None]
        iy *= scy
        iy += oy
        x0 = np.floor(ix)
        y0 = np.floor(iy)
        wx = ix - x0
        wy = iy - y0
        xi = x0.astype(np.int32)
        xi += padb
        np.clip(xi, 0, Wp - 2, out=xi)
        yi = y0.astype(np.int32)
        yi += padb
        np.clip(yi, 0, Wp - 2, out=yi)
        base = yi
        base *= Wp
        base += xi
        b00 = flat[base]
        b01 = flat[base + 1]
        b10 = flat[base + Wp]
        b11 = flat[base + Wp + 1]
        top = (1.0 - wx) * b00
        top += wx * b01
        bot = (1.0 - wx) * b10
        bot += wx * b11
        val = (1.0 - wy) * top
        val += wy * bot
        out[s0 : s0 + n] = val @ ego_f
    return out


def _theta_warp_fields(base_2x3, dr, align_corners):
    """Pixel-coord sample positions of the theta-only warp (dx=dy=0)."""
    th = np.float32(dr) * np.float32(math.pi / 180.0)
    c, s = np.cos(th, dtype=np.float32), np.sin(th, dtype=np.float32)
    delta = np.array([[c, -s, 0], [s, c, 0], [0, 0, 1]], np.float32)
    base3 = np.concatenate([base_2x3, [[0, 0, 1]]], 0).astype(np.float32)
    aff = (base3 @ delta)[:2]
    xs, ys = _grid_1d(align_corners)
    gx = aff[0, 0] * xs[None, :] + aff[0, 1] * ys[:, None] + aff[0, 2]
    gy = aff[1, 0] * xs[None, :] + aff[1, 1] * ys[:, None] + aff[1, 2]
    if align_corners:
        ix = (gx + 1.0) * (0.5 * (W - 1))
        iy = (gy + 1.0) * (0.5 * (H - 1))
    else:
        ix = gx * (0.5 * W) + (0.5 * W - 0.5)
        iy = gy * (0.5 * H) + (0.5 * H - 0.5)
    return ix.astype(np.float32), iy.astype(np.float32)


def _trans_shifts(base_2x3, cands, align_corners):
    """Pixel-space shifts (ux, uy) each candidate translation adds."""
    B2 = base_2x3[:2, :2].astype(np.float32)
    tx = (2.0 * cands[:, 0] / (W - 1)).astype(np.float32)
    ty = (2.0 * cands[:, 1] / (H - 1)).astype(np.float32)
    if align_corners:
        sx, sy = 0.5 * (W - 1), 0.5 * (H - 1)
    else:
        sx, sy = 0.5 * W, 0.5 * H
    ux = (B2[0, 0] * tx + B2[0, 1] * ty) * np.float32(sx)
    uy = (B2[1, 0] * tx + B2[1, 1] * ty) * np.float32(sy)
    return ux, uy


def _build_splats(ego_c, ix, iy):
    """Moment splat canvases [4, CY, CX] f32, or None if out of range."""
    Xi = np.floor(ix)
    Yi = np.floor(iy)
    Xf = (ix - Xi).astype(np.float32)
    Yf = (iy - Yi).astype(np.float32)
    Xi = Xi.astype(np.int64)
    Yi = Yi.astype(np.int64)
    if (
        Xi.min() < -OFFX
        or Xi.max() >= CX - OFFX
        or Yi.min() < -OFFY
        or Yi.max() >= CY - OFFY
    ):
        return None
    S = np.zeros((4, CY, CX), np.float32)
    flatidx = ((Yi + OFFY) * CX + (Xi + OFFX)).ravel()
    nbins = CY * CX
    for m, mu in enumerate((None, Xf, Yf, Xf * Yf)):
        wgt = ego_c if mu is None else mu * ego_c
        S[m] = (
            np.bincount(flatidx, weights=wgt.ravel().astype(np.float64), minlength=nbins)
            .reshape(CY, CX)
            .astype(np.float32)
        )
    return S


def _build_wq(nbr_c):
    """Stationary windows WQ[c, i, h, 2t+slot] = nbr[2i+slot, c+128h+t-68]."""
    WQ = np.zeros((128, 64, 2, MROWS), np.float32)
    c = np.arange(128)[:, None, None]
    h = np.arange(2)[None, :, None]
    t = np.arange(NL)[None, None, :]
    v = c + 128 * h + t - 68
    valid = (v >= 0) & (v < W)
    vc = np.clip(v, 0, W - 1)
    for slot in range(2):
        # [c, h, t] gather per row y -> place at [c, i, h, slot::2]
        rows = nbr_c[slot::2, :]  # [64, W]
        vals = np.where(valid[None], rows[:, vc], 0.0)  # [64, c, h, t]
        WQ[:, :, :, slot::2] = vals.transpose(1, 0, 2, 3)
    return WQ


def _assemble_approx(T, base_2x3, cands, align_corners):
    """Approx scores for one theta's candidates from its surface T [NL, 4, NL].

    Returns None if any candidate's lag falls outside the computed window
    (caller falls back to the exact host path)."""
    ux, uy = _trans_shifts(base_2x3, cands, align_corners)
    Ui = np.floor(ux).astype(np.int64)
    Ufx = (ux - Ui).astype(np.float32)
    Vi = np.floor(uy).astype(np.int64)
    Ufy = (uy - Vi).astype(np.float32)
    if (
        Ui.min() < LMIN
        or Ui.max() + 1 >= LMIN + NL
        or Vi.min() < LMIN
        or Vi.max() + 1 >= LMIN + NL
    ):
        return None
    out = np.zeros(len(cands), np.float32)
    for j in (0, 1):
        ay = np.where(j, Ufy, 1.0 - Ufy).astype(np.float32)
        by = 1.0 if j else -1.0
        Jp = Vi + j - LMIN
        for k in (0, 1):
            ax = np.where(k, Ufx, 1.0 - Ufx).astype(np.float32)
            bx = 1.0 if k else -1.0
            Kp = Ui + k - LMIN
            out += ax * ay * T[Kp, 0, Jp]
            out += bx * ay * T[Kp, 1, Jp]
            out += ax * by * T[Kp, 2, Jp]
            out += bx * by * T[Kp, 3, Jp]
    return out


def _combine_T(psum):
    """psum [MROWS, NJ, 2, 4] -> T[u][K(NL), m(4), J(NL)]."""
    T = np.zeros((2, NL, 4, NL), np.float32)
    J = np.arange(LMIN, LMIN + NL)
    w0 = 23 - J
    w1 = 24 - J
    for u in range(2):
        p0 = psum[0::2, :, u, :][:, w0, :]  # [t, J, m]
        p1 = psum[1::2, :, u, :][:, w1, :]
        T[u] = (p0 + p1).transpose(0, 2, 1)
    return T


# ----------------------------------------------------------------------------
# device program
# ----------------------------------------------------------------------------

WQ_CHUNKS = (8, 16, 16, 24)      # i-axis chunk sizes (first small: MM0 dep)
SR_CHUNKS = (64, 56, 56)         # y'-axis chunk sizes (chunk0 covers i<8)
N_WARMUP_MM = 5                  # 512-col dummies bridge PE-start -> DMA-ready;
                                 # the first few real MMs finish the HAM warmup


def _get_nc():
    global _NC
    if _NC is not None:
        return _NC
    sys.path.insert(0, "/opt/trn_rl_repo")
    from contextlib import ExitStack

    import concourse.bass as bass
    import concourse.mybir as mybir
    import concourse.tile as tile
    from concourse import bacc

    nc = bacc.Bacc("TRN2", target_bir_lowering=False, debug=False)
    wq = nc.declare_dram_parameter("wq", [128, 64, 2, MROWS], mybir.dt.float8e4, isOutput=False)
    sra = nc.declare_dram_parameter("sra", [128, 2, SRY, 8], mybir.dt.float8e4, isOutput=False)
    srb = nc.declare_dram_parameter("srb", [128, 2, SRY, 8], mybir.dt.float8e4, isOutput=False)
    tout = nc.declare_dram_parameter("tout", [NGRP, MROWS, NCOLS], mybir.dt.float32, isOutput=True)
    wq_h = wq.tensor if isinstance(wq, bass.AP) else wq
    sra_h = sra.tensor if isinstance(sra, bass.AP) else sra
    srb_h = srb.tensor if isinstance(srb, bass.AP) else srb
    tout_h = tout.tensor if isinstance(tout, bass.AP) else tout

    DR = mybir.MatmulPerfMode.DoubleRow

    with ExitStack() as ctx:
        tc = ctx.enter_context(tile.TileContext(nc))
        pool = ctx.enter_context(tc.tile_pool(name="persist", bufs=1))
        psum_pool = ctx.enter_context(tc.tile_pool(name="psum", bufs=1, space="PSUM"))

        wq_t = pool.tile([128, 64, 2, MROWS], mybir.dt.float8e4)
        sra_t = pool.tile([128, 2, SRY, 8], mybir.dt.float8e4)
        srb_t = pool.tile([128, 2, SRY, 8], mybir.dt.float8e4)

        # PE warm-up: zero-filled dummy DoubleRow matmuls keep the HAM busy
        # while input DMAs land, so the real loop starts at 2.4 GHz.
        wdum = pool.tile([128, 2, 16], mybir.dt.float8e4)
        sdum = pool.tile([128, 2, 512], mybir.dt.float8e4)
        pdum = psum_pool.tile([16, 512], mybir.dt.float32, name="pdum", tag="pdum")
        nc.vector.memset(wdum[:], 0.0)
        nc.vector.memset(sdum[:], 0.0)
        for k in range(N_WARMUP_MM):
            nc.tensor.matmul(pdum[:], wdum[:], sdum[:], start=True, stop=True,
                             perf_mode=DR, skip_group_check=True)

        # chunked input DMAs, split across both HWDGE issuing engines so the
        # ~0.6us per-descriptor issue cost parallelizes; each engine's first
        # descriptor covers MM0's dependencies so the PE loop starts early.
        def wq_src(i0, csz):
            return bass.AP(tensor=wq_h, offset=i0 * 2 * MROWS,
                           ap=[[64 * 2 * MROWS, 128], [1, csz * 2 * MROWS]])

        def sr_src(srh, y0, y1):
            return bass.AP(tensor=srh, offset=y0 * 8,
                           ap=[[2 * SRY * 8, 128], [SRY * 8, 2], [1, (y1 - y0) * 8]])

        wq_offs = [sum(WQ_CHUNKS[:k]) for k in range(len(WQ_CHUNKS))]
        sr_offs = [sum(SR_CHUNKS[:k]) for k in range(len(SR_CHUNKS))]
        sync_plan = [("wq", 0), ("srb", 0), ("wq", 1), ("wq", 2), ("wq", 3)]
        scal_plan = [("sra", 0), ("sra", 1), ("srb", 1), ("sra", 2), ("srb", 2)]
        for eng, plan in ((nc.sync, sync_plan), (nc.scalar, scal_plan)):
            for kind, k in plan:
                if kind == "wq":
                    eng.dma_start(out=wq_t[:, wq_offs[k]:wq_offs[k] + WQ_CHUNKS[k]],
                                  in_=wq_src(wq_offs[k], WQ_CHUNKS[k]))
                else:
                    srh, srt = (sra_h, sra_t) if kind == "sra" else (srb_h, srb_t)
                    y0, y1 = sr_offs[k], sr_offs[k] + SR_CHUNKS[k]
                    eng.dma_start(out=srt[:, :, y0:y1], in_=sr_src(srh, y0, y1))

        psums = [
            psum_pool.tile([MROWS, NCOLS], mybir.dt.float32, name=f"psum{g}", tag=f"psum{g}")
            for g in range(NGRP)
        ]
        for i in range(64):
            lhsT = wq_t[:, i]  # [128, 2, MROWS]
            for g, srt in enumerate((sra_t, srb_t)):
                rhs = srt[:, :, 2 * i:2 * i + NJ, :]  # [128, 2, NJ, 8]
                nc.tensor.matmul(psums[g][:], lhsT, rhs,
                                 start=(i == 0), stop=(i == 63), perf_mode=DR)

        for g in range(NGRP):
            stg = pool.tile([MROWS, NCOLS], mybir.dt.float32, name=f"stg{g}", tag=f"stg{g}")
            nc.vector.tensor_copy(stg[:], psums[g][:])
            dst = bass.AP(tensor=tout_h, offset=g * MROWS * NCOLS,
                          ap=[[NCOLS, MROWS], [1, NCOLS]])
            eng = nc.sync if g == 0 else nc.scalar
            eng.dma_start(out=dst, in_=stg[:])
    nc.compile()
    _NC = nc
    return nc


def _run_device(in_maps, trace=False):
    sys.path.insert(0, "/opt/trn_rl_repo")
    import ml_dtypes
    from concourse.bass_utils import run_bass_kernel_spmd

    fp8 = ml_dtypes.float8_e4m3
    maps = [
        {
            "wq": np.ascontiguousarray(m["wq"]).astype(fp8),
            "sra": np.ascontiguousarray(m["sra"]).astype(fp8),
            "srb": np.ascontiguousarray(m["srb"]).astype(fp8),
        }
        for m in in_maps
    ]
    res = run_bass_kernel_spmd(_get_nc(), maps, core_ids=list(range(len(maps))),
                               trace=trace)
    touts = [
        r["tout"].astype(np.float32).reshape(NGRP, MROWS, NJ, 2, 4) for r in res.results
    ]
    if trace:
        return touts, res
    return touts


# ----------------------------------------------------------------------------
# pipeline
# ----------------------------------------------------------------------------

def _refine_pair_host_only(ego_c, nbr_c, base, align_corners):
    """Pure-host exact fallback (pathological inputs only)."""
    nbrP = _pad_nbr(nbr_c)
    cands = _coarse_cands()
    sc = _exact_scores(ego_c, nbrP, _cand_affines(cands, base), align_corners)
    bi = int(np.argmax(sc))
    cp = cands[bi] if sc[bi] > 1e-5 else np.zeros(3, np.float32)
    if np.all(cp == 0.0):
        return base
    fc = _fine_cands(cp)
    affs_f = _cand_affines(fc, base)
    sf = _exact_scores(ego_c, nbrP, affs_f, align_corners)
    bif = int(np.argmax(sf))
    return affs_f[bif] if sf[bif] > 1e-5 else base


def _finish_pair(ego_c, nbrP, base, cands, approx, align_corners):
    """Adaptive exact rescore of the approx-selected coarse set -> cp."""
    thresh = approx.max() - DELTA_COARSE
    sel = np.where(approx >= thresh)[0]
    if len(sel) > RESCORE_CAP:
        sel = sel[np.argsort(approx[sel])[::-1][:RESCORE_CAP]]
    if len(sel) < 48:
        sel = np.argsort(approx)[::-1][:48]
    affs = _cand_affines(cands[sel], base)
    sc = _exact_scores(ego_c, nbrP, affs, align_corners)
    bi_local = int(np.argmax(sc))
    bi = int(sel[bi_local])
    ok = sc[bi_local] > 1e-5
    cp = cands[bi] if ok else np.zeros(3, np.float32)
    return cp


def kernel(occ_map, record_len, affine_matrix, align_corners):
    occ = np.asarray(occ_map, dtype=np.float32)
    rl = np.asarray(record_len).reshape(-1)
    aff_in = np.asarray(affine_matrix)
    out_dtype = aff_in.dtype
    refined = aff_in.astype(np.float32).copy()
    ac = bool(np.asarray(align_corners))

    # pair list exactly as the reference builds it
    pairs = []
    idx = 0
    for b in range(len(rl)):
        n_agents = int(rl[b])
        grp0 = idx
        idx += n_agents
        if n_agents <= 1:
            continue
        for n in range(1, n_agents):
            pairs.append((b, n, grp0, grp0 + n))
    if not pairs:
        return refined.astype(out_dtype)

    device_ok = (
        len(pairs) <= 2
        and all(
            b < refined.shape[0] and n < refined.shape[2] and nb < occ.shape[0]
            for (b, n, _, nb) in pairs
        )
    )

    pair_data = []
    for (b, n, ei, ni) in pairs:
        # mimic jax OOB semantics: clip gather indices, drop OOB scatters
        ei = min(ei, occ.shape[0] - 1)
        ni = min(ni, occ.shape[0] - 1)
        ego = occ[ei, 0]
        nbr = occ[ni, 0]
        ego_c = np.where(ego > THRESH, ego, 0.0).astype(np.float32)
        nbr_c = np.where(nbr > THRESH, nbr, 0.0).astype(np.float32)
        base = refined[b, 0, n].astype(np.float32)
        pair_data.append(
            {
                "b": min(b, refined.shape[0] - 1),
                "n": n,
                "ego_c": ego_c,
                "nbr_c": nbr_c,
                "nbrP": _pad_nbr(nbr_c),
                "base": base,
            }
        )

    cands = _coarse_cands()
    drs = np.unique(cands[:, 2])  # 16 rotations
    by_dr = {float(dr): np.where(cands[:, 2] == dr)[0] for dr in drs}

    # build device inputs: 16 theta-units per pair, 4 per core; cores 0-3 pair0,
    # cores 4-7 pair1
    use_device = device_ok
    unit_map = {}  # (core, u) -> (pair_idx, dr)
    in_maps = None
    if use_device:
        zero_wq = np.zeros((128, 64, 2, MROWS), np.float32)
        zero_sr = np.zeros((128, 2, SRY, 8), np.float32)
        in_maps = []
        splat_fail = False
        wq_cache = {}
        for core in range(N_CORES):
            pi = core // 4
            if pi >= len(pair_data):
                in_maps.append({"wq": zero_wq, "sra": zero_sr, "srb": zero_sr})
                continue
            pd = pair_data[pi]
            if pi not in wq_cache:
                wq_cache[pi] = _build_wq(pd["nbr_c"])
            srs = []
            for g in range(NGRP):
                SR = np.zeros((128, 2, SRY, 8), np.float32)
                for ug in range(2):
                    u = 2 * g + ug
                    th_idx = 4 * (core % 4) + u
                    dr = float(drs[th_idx])
                    ix, iy = _theta_warp_fields(pd["base"], dr, ac)
                    S = _build_splats(pd["ego_c"], ix, iy)
                    if S is None:
                        splat_fail = True
                        break
                    # SR[c, h, y', u, m] = S[m, SRY0 + y', c + 128h]
                    blk = S[:, SRY0:SRY0 + SRY, :].transpose(2, 1, 0)  # [cx, y', m]
                    SR[:, :, :, 4 * ug:4 * ug + 4] = (
                        blk.reshape(2, 128, SRY, 4).transpose(1, 0, 2, 3)
                    )
                    unit_map[(core, u)] = (pi, dr)
                if splat_fail:
                    break
                srs.append(SR)
            if splat_fail:
                break
            in_maps.append({"wq": wq_cache[pi], "sra": srs[0], "srb": srs[1]})
        if splat_fail:
            use_device = False

    if use_device:
        try:
            global _LAST_IN_MAPS
            _LAST_IN_MAPS = in_maps
            touts = _run_device(in_maps)
        except Exception:
            use_device = False

    for pi, pd in enumerate(pair_data):
        base = pd["base"]
        pair_device = use_device
        approx = None
        if pair_device:
            approx = np.empty(len(cands), np.float32)
            for core in range(4 * pi, 4 * pi + 4):
                for g in range(NGRP):
                    Tg = _combine_T(touts[core][g])
                    for ug in range(2):
                        u = 2 * g + ug
                        key = (core, u)
                        if key not in unit_map:
                            continue
                        _, dr = unit_map[key]
                        sel = by_dr[dr]
                        a = _assemble_approx(Tg[ug], base, cands[sel], ac)
                        if a is None:
                            pair_device = False
                            break
                        approx[sel] = a
                    if not pair_device:
                        break
                if not pair_device:
                    break
        if pair_device:
            cp = _finish_pair(pd["ego_c"], pd["nbrP"], base, cands, approx, ac)
            if np.all(cp == 0.0):
                new_aff = base
            else:
                fc = _fine_cands(cp)
                affs_f = _cand_affines(fc, base)
                sf = _exact_scores(pd["ego_c"], pd["nbrP"], affs_f, ac)
                bif = int(np.argmax(sf))
                new_aff = affs_f[bif] if sf[bif] > 1e-5 else base
        else:
            new_aff = _refine_pair_host_only(pd["ego_c"], pd["nbr_c"], base, ac)
        if pd["n"] < refined.shape[2] and pd["b"] < refined.shape[0]:
            refined[pd["b"], 0, pd["n"]] = new_aff

    return refined.astype(out_dtype)
